# revision 2
# baseline (speedup 1.0000x reference)
"""Trainium2 Bass kernel for a 4-layer GPT language model.

Model: B=2, T=2048, C=512, H=8 heads, L=4 layers, V=32000, relative-position
bias (33 buckets, clip +-16), causal attention, ReLU FFN (4C hidden),
final LN + untied output projection.

Sharding over 8 NeuronCores (one uniform SPMD program; all per-core
differences live in the input data):
 - attention: head-parallel (core r computes head r for all tokens/batches)
 - LN / residual / FFN / Wo projection: token-parallel (core r owns the 512
   contiguous tokens of batch r//4, block r%4)
 - lm_head: vocab-parallel (core r computes Wout columns [4000r, 4000(r+1)))
 - per layer: one AllGather of post-LN1 activations (transposed, bf16) and
   one AllToAll of attention head outputs (delivers all heads for own tokens
   at uniform offsets); one more AllGather before the lm_head.

All matmul operands bf16 with fp32 PSUM accumulation; LN/softmax/residual in
fp32. Softmax runs without max-subtraction (scores are O(1) at this scale).
Causal mask + RPE bias are applied as one host-precomputed [128,512] add on
the pre-exp scores: -1e30 in the masked triangle, RPE delta vs rpe[l,0,h] in
the 17-wide diagonal band (softmax shift-invariance removes the constant).
The embedding gather, bf16 casts and layout packing happen on host; the host
reassembles the vocab-sharded per-core outputs.
"""

import sys

for _p in ("/opt/trn_rl_repo", "/root/.axon_site/_ro/trn_rl_repo"):
    if _p not in sys.path:
        sys.path.append(_p)

import numpy as np
import ml_dtypes

import concourse.bass as bass
import concourse.bacc as bacc
import concourse.mybir as mybir
import concourse.tile as tile
from concourse import masks
from concourse.bass_utils import run_bass_kernel_spmd

BF16 = ml_dtypes.bfloat16

# model dims
B, C, H, L, V, MD = 2, 512, 8, 4, 32000, 16
D = C // H          # 64
HID = 4 * C         # 2048
NB = 2 * MD + 1     # 33
NCORES = 8
CC = C // 128       # 4 c-chunks
NHS = HID // 128    # 16 hidden slices
NEG = -1.0e30

F32 = mybir.dt.float32
BF = mybir.dt.bfloat16


def build_program(n_tt=4, vsh=4000, n_cores=NCORES):
    """Build the uniform per-core program.

    n_tt: token tiles of 128 owned per core (4 -> T=2048 per batch)
    vsh:  vocab columns owned per core
    """
    t_own = 128 * n_tt            # tokens owned per core
    t_batch = 4 * t_own           # tokens per batch (4 cores per batch)
    t_glob = 2 * t_batch          # tokens across both batches
    n_qc = t_batch // 512         # 512-wide query chunks per batch
    n_kt = t_batch // 128         # 128-wide key tiles per batch
    n_vt = t_glob // 128          # V tiles (both batches)
    vc_w = min(vsh, 500)          # vocab chunk width
    n_vc = (vsh + vc_w - 1) // vc_w
    rg = [list(range(n_cores))]
    AF = mybir.ActivationFunctionType
    ALU = mybir.AluOpType

    nc = bacc.Bacc("TRN2", target_bir_lowering=False, debug=False,
                   num_devices=n_cores)

    # ---- per-core external inputs (host pre-packed, see _shard_inputs) ----
    x0 = nc.dram_tensor("x0", [t_own, C], F32, kind="ExternalInput")
    wqkv_d = nc.dram_tensor("wqkv", [L, 128, 3, CC, D], BF, kind="ExternalInput")
    wo_d = nc.dram_tensor("wo", [L, 128, CC, C], BF, kind="ExternalInput")
    w1_d = nc.dram_tensor("w1", [L, 128, CC, HID], BF, kind="ExternalInput")
    w2_d = nc.dram_tensor("w2", [L, 128, NHS, C], BF, kind="ExternalInput")
    wout_d = nc.dram_tensor("wout", [128, CC, vsh], BF, kind="ExternalInput")
    b1t_d = nc.dram_tensor("b1t", [L, 128, NHS], F32, kind="ExternalInput")
    # replicated per-column vectors, order: g1, be1, g2, be2, bo, b2
    ln_rep = nc.dram_tensor("ln_rep", [L, 128, 6, C], F32, kind="ExternalInput")
    lnf_rep = nc.dram_tensor("lnf_rep", [128, 2, C], F32, kind="ExternalInput")
    bout_rep = nc.dram_tensor("bout_rep", [128, vsh], F32, kind="ExternalInput")
    # combined causal-mask + RPE-delta tiles per diagonal offset oi = (k0-q0)/128
    mb_d = nc.dram_tensor("maskband", [L, 128, 4, 512], F32, kind="ExternalInput")
    # spill band for the k-tile just before a query chunk (cols q-k0 in [128,144))
    spill_d = nc.dram_tensor("spill", [L, 128, 16], F32, kind="ExternalInput")

    y = nc.dram_tensor("y", [t_glob, vsh], F32, kind="ExternalOutput")

    with tile.TileContext(nc) as tc:
        pools = []

        def pool(**kw):
            cm = tc.tile_pool(**kw)
            p = cm.__enter__()
            pools.append(cm)
            return p

        const = pool(name="const", bufs=1)
        persist = pool(name="persist", bufs=1)
        xtc_pool = pool(name="xtc", bufs=3)   # AG'd xT chunks
        small = pool(name="small", bufs=4)    # [128,1] LN scalars
        rows = pool(name="rows", bufs=2)      # [1,512] denom rows
        scratch = pool(name="scratch", bufs=2)
        olhs = pool(name="olhs", bufs=3)
        ppool = pool(name="ppool", bufs=4)    # P (exp scores) tiles
        ysb_pool = pool(name="ysb", bufs=4)

        ps_t = pool(name="ps_t", bufs=1, space="PSUM")
        ps_proj = pool(name="ps_proj", bufs=2, space="PSUM")
        ps_s_cm = tc.tile_pool(name="ps_s", bufs=2, space="PSUM")
        ps_s = ps_s_cm.__enter__()
        ps_o_cm = tc.tile_pool(name="ps_o", bufs=2, space="PSUM")
        ps_o = ps_o_cm.__enter__()
        ps_bc_cm = tc.tile_pool(name="ps_bc", bufs=1, space="PSUM")
        ps_bc = ps_bc_cm.__enter__()

        dram = pool(name="dram", bufs=2, space="DRAM")

        lay_cm = tc.tile_pool(name="lay", bufs=1)
        lay = lay_cm.__enter__()
        wpool_cm = tc.tile_pool(name="wpool", bufs=1)
        wpool = wpool_cm.__enter__()

        # ---------------- constants ----------------
        ident = const.tile([128, 128], BF)
        masks.make_identity(nc, ident[:])
        ones_row = const.tile([1, 64], BF)
        nc.vector.memset(ones_row[:], 1.0)
        eps_t = const.tile([128, 1], F32)
        nc.vector.memset(eps_t[:], 1e-5)

        # residual stream, token-major fp32; tile tt at cols [tt*C,(tt+1)*C)
        x_sb = persist.tile([128, n_tt * C], F32)
        nc.sync.dma_start(
            x_sb[:].rearrange("p (tt c) -> p tt c", tt=n_tt),
            x0.ap().rearrange("(tt p) c -> p tt c", p=128))

        def layernorm(dst_ap, src_ap, g_ap, b_ap):
            """dst(bf16) = LN(src) with replicated gamma/beta; src [128,C] f32."""
            ssum = small.tile([128, 1], F32, tag="ln_ssum")
            nc.vector.tensor_reduce(ssum[:], src_ap, mybir.AxisListType.X,
                                    ALU.add)
            nmean = small.tile([128, 1], F32, tag="ln_nmean")
            nc.scalar.mul(nmean[:], ssum[:], -1.0 / C)
            xc = scratch.tile([128, C], F32, tag="ln_xc")
            nc.scalar.activation(xc[:], src_ap, AF.Identity, bias=nmean[:])
            sq = scratch.tile([128, C], F32, tag="ln_sq")
            vsum = small.tile([128, 1], F32, tag="ln_vsum")
            nc.scalar.activation(sq[:], xc[:], AF.Square, accum_out=vsum[:])
            std = small.tile([128, 1], F32, tag="ln_std")
            nc.scalar.activation(std[:], vsum[:], AF.Sqrt, bias=eps_t[:],
                                 scale=1.0 / C)
            rstd = small.tile([128, 1], F32, tag="ln_rstd")
            nc.vector.reciprocal(rstd[:], std[:])
            tmp = scratch.tile([128, C], F32, tag="ln_tmp")
            nc.vector.scalar_tensor_tensor(tmp[:], xc[:], rstd[:], g_ap,
                                           ALU.mult, ALU.mult)
            nc.vector.tensor_add(dst_ap, tmp[:], b_ap)

        def transpose_128(dst_ap, src_ap):
            """dst[128,128] = src[128,128].T via PE; bf16 in/out."""
            pt = ps_t.tile([128, 128], BF, tag="ps_t")
            nc.tensor.transpose(pt[:], src_ap, ident[:])
            nc.vector.tensor_copy(dst_ap, pt[:])

        def ln_transpose_ag(g_ap, b_ap, tag, xt_pool):
            """LN own tiles -> xT_own bf16 [128, CC*t_own] (cc-major), AG it."""
            xT_own = xt_pool.tile([128, CC * t_own], BF, tag=f"xT_{tag}")
            for tt in range(n_tt):
                xln = scratch.tile([128, C], BF, tag="ln_out")
                layernorm(xln[:], x_sb[:, tt * C:(tt + 1) * C], g_ap, b_ap)
                for cc in range(CC):
                    transpose_128(
                        xT_own[:, cc * t_own + tt * 128:
                               cc * t_own + (tt + 1) * 128],
                        xln[:, cc * 128:(cc + 1) * 128])
            bounce = dram.tile([128, CC * t_own], BF, tag="bnc_x")
            nc.sync.dma_start(bounce[:], xT_own[:])
            ag_out = dram.tile([n_cores, 128, CC * t_own], BF,
                               addr_space="Shared", tag="ag_x")
            nc.gpsimd.collective_compute(
                "AllGather", ALU.bypass, replica_groups=rg,
                ins=[bounce[:].opt()], outs=[ag_out[:].opt()])
            return ag_out

        # ================= transformer layers =================
        for l in range(L):
            lrep = wpool.tile([128, 6 * C], F32, tag="lrep")
            nc.sync.dma_start(
                lrep[:].rearrange("p (k c) -> p k c", k=6), ln_rep.ap()[l])
            g1, be1 = lrep[:, 0 * C:1 * C], lrep[:, 1 * C:2 * C]
            g2, be2 = lrep[:, 2 * C:3 * C], lrep[:, 3 * C:4 * C]
            bo_r, b2_r = lrep[:, 4 * C:5 * C], lrep[:, 5 * C:6 * C]

            # ---- LN1 -> transpose -> AllGather ----
            ag_x = ln_transpose_ag(g1, be1, "x1", lay)

            # ---- QKV projections over all gathered token chunks ----
            wq_t = wpool.tile([128, 3 * CC * D], BF, tag="wqkv")
            nc.sync.dma_start(
                wq_t[:].rearrange("p (k cc d) -> p k cc d", k=3, cc=CC),
                wqkv_d.ap()[l])

            def wslice(k, cc):
                base = k * CC * D + cc * D
                return wq_t[:, base:base + D]

            qf = lay.tile([64, t_glob], BF, tag="qf")
            kf = lay.tile([64, t_glob], BF, tag="kf")
            vaug = lay.tile([128, n_vt * 65], BF, tag="vaug")
            nc.vector.memset(
                vaug[:].rearrange("p (n e) -> p n e", e=65)[:, :, 64:65], 1.0)

            for r in range(n_cores):
                xtc = xtc_pool.tile([128, CC * t_own], BF, tag="xtc")
                nc.sync.dma_start(xtc[:], ag_x[r])
                cols = slice(r * t_own, (r + 1) * t_own)
                for k, dst, scale in ((0, qf, float(1.0 / np.sqrt(D))),
                                      (1, kf, 1.0)):
                    pq = ps_proj.tile([64, t_own], F32, tag="mm")
                    for cc in range(CC):
                        nc.tensor.matmul(
                            pq[:], wslice(k, cc),
                            xtc[:, cc * t_own:(cc + 1) * t_own],
                            start=(cc == 0), stop=(cc == CC - 1))
                    nc.scalar.mul(dst[:, cols], pq[:], scale)
                for st in range(n_tt):
                    pv = ps_proj.tile([128, D], F32, tag="mm")
                    for cc in range(CC):
                        nc.tensor.matmul(
                            pv[:],
                            xtc[:, cc * t_own + st * 128:
                                cc * t_own + (st + 1) * 128],
                            wslice(2, cc), start=(cc == 0), stop=(cc == CC - 1))
                    vt = r * n_tt + st
                    nc.vector.tensor_copy(vaug[:, vt * 65: vt * 65 + 64],
                                          pv[:])

            # ---- attention (transposed scores [k,q], streaming) ----
            mb = wpool.tile([128, 4 * 512], F32, tag="mb")
            nc.sync.dma_start(
                mb[:].rearrange("p (oi j) -> p oi j", oi=4), mb_d.ap()[l])
            spill = wpool.tile([128, 16], F32, tag="spill")
            nc.sync.dma_start(spill[:], spill_d.ap()[l])
            of = lay.tile([64, t_glob], BF, tag="of")

            for b in range(2):
                for qc in range(n_qc):
                    q0 = qc * 512
                    qcols = slice(b * t_batch + q0, b * t_batch + q0 + 512)
                    po = ps_o.tile([65, 512], F32, tag="ps_o")
                    nkt = 4 * (qc + 1)
                    for kt in range(nkt):
                        k0 = kt * 128
                        ps = ps_s.tile([128, 512], F32, tag="ps_s")
                        nc.tensor.matmul(
                            ps[:],
                            kf[:, b * t_batch + k0: b * t_batch + k0 + 128],
                            qf[:, qcols], start=True, stop=True)
                        if k0 >= q0:      # diagonal region: mask + RPE band
                            oi = (k0 - q0) // 128
                            nc.vector.tensor_add(
                                ps[:], ps[:], mb[:, oi * 512:(oi + 1) * 512])
                        elif k0 == q0 - 128:  # band spill from prev chunk
                            nc.vector.tensor_add(ps[:, 0:16], ps[:, 0:16],
                                                 spill[:])
                        p_t = ppool.tile([128, 512], BF, tag="p")
                        nc.scalar.activation(p_t[:], ps[:], AF.Exp)
                        nc.tensor.matmul(
                            po[:],
                            vaug[:, (b * n_kt + kt) * 65:
                                 (b * n_kt + kt) * 65 + 65],
                            p_t[:], start=(kt == 0), stop=(kt == nkt - 1))
                    # normalize: 1/denom row broadcast via K=1 matmul
                    den = rows.tile([1, 512], F32, tag="den")
                    nc.vector.tensor_copy(den[:], po[64:65, :])
                    rden = rows.tile([1, 512], F32, tag="rden")
                    nc.vector.reciprocal(rden[:], den[:])
                    rb = rows.tile([1, 512], BF, tag="rb")
                    nc.vector.tensor_copy(rb[:], rden[:])
                    pbc = ps_bc.tile([64, 512], F32, tag="ps_bc")
                    nc.tensor.matmul(pbc[:], ones_row[:], rb[:],
                                     start=True, stop=True)
                    bc_sb = scratch.tile([64, 512], F32, tag="bc_sb")
                    nc.scalar.copy(bc_sb[:], pbc[:])
                    nc.vector.tensor_mul(of[:, qcols], po[0:64, :], bc_sb[:])

            # ---- AllToAll: send per-token-owner chunks of own head's o,
            # receive all heads' o for own tokens -> [C, t_own] feature-major
            bounce_o = dram.tile([n_cores, 64, t_own], BF, tag="bnc_o")
            for r in range(n_cores):
                nc.sync.dma_start(bounce_o[r],
                                  of[:, r * t_own:(r + 1) * t_own])
            o_own = dram.tile([n_cores * 64, t_own], BF, tag="a2a_o")
            nc.gpsimd.collective_compute(
                "AllToAll", ALU.bypass, replica_groups=rg,
                ins=[bounce_o[:].opt()], outs=[o_own[:].opt()])

            # ---- Wo projection on own tokens + residual ----
            wo_t = wpool.tile([128, CC * C], BF, tag="wo")
            nc.sync.dma_start(
                wo_t[:].rearrange("p (cc c) -> p cc c", cc=CC), wo_d.ap()[l])
            for tt in range(n_tt):
                px = ps_proj.tile([128, C], F32, tag="mm")
                for cc in range(CC):
                    ot = olhs.tile([128, 128], BF, tag="o_lhsT")
                    nc.sync.dma_start(
                        ot[:], o_own[cc * 128:(cc + 1) * 128,
                                     tt * 128:(tt + 1) * 128])
                    nc.tensor.matmul(px[:], ot[:],
                                     wo_t[:, cc * C:(cc + 1) * C],
                                     start=(cc == 0), stop=(cc == CC - 1))
                xt = x_sb[:, tt * C:(tt + 1) * C]
                nc.vector.scalar_tensor_tensor(xt, px[:], 1.0, xt,
                                               ALU.mult, ALU.add)
                nc.vector.tensor_add(xt, xt, bo_r)

            # ---- LN2 -> transpose (no AG; FFN is token-local) ----
            xT2 = lay.tile([128, CC * t_own], BF, tag="xT2")
            for tt in range(n_tt):
                xln = scratch.tile([128, C], BF, tag="ln_out")
                layernorm(xln[:], x_sb[:, tt * C:(tt + 1) * C], g2, be2)
                for cc in range(CC):
                    transpose_128(
                        xT2[:, cc * t_own + tt * 128:
                            cc * t_own + (tt + 1) * 128],
                        xln[:, cc * 128:(cc + 1) * 128])

            # ---- FFN ----
            w1_t = wpool.tile([128, CC * HID], BF, tag="w1")
            nc.sync.dma_start(
                w1_t[:].rearrange("p (cc j) -> p cc j", cc=CC), w1_d.ap()[l])
            b1_t = wpool.tile([128, NHS], F32, tag="b1t")
            nc.sync.dma_start(b1_t[:], b1t_d.ap()[l])
            actsT = lay.tile([128, NHS * t_own], BF, tag="actsT")
            for hs in range(NHS):
                ph = ps_proj.tile([128, t_own], F32, tag="mm")
                for cc in range(CC):
                    nc.tensor.matmul(
                        ph[:],
                        w1_t[:, cc * HID + hs * 128: cc * HID + (hs + 1) * 128],
                        xT2[:, cc * t_own:(cc + 1) * t_own],
                        start=(cc == 0), stop=(cc == CC - 1))
                nc.scalar.activation(
                    actsT[:, hs * t_own:(hs + 1) * t_own], ph[:], AF.Relu,
                    bias=b1_t[:, hs:hs + 1])

            w2_t = wpool.tile([128, NHS * C], BF, tag="w2")
            nc.sync.dma_start(
                w2_t[:].rearrange("p (hs c) -> p hs c", hs=NHS), w2_d.ap()[l])
            for tt in range(n_tt):
                pf = ps_proj.tile([128, C], F32, tag="mm")
                for hs in range(NHS):
                    nc.tensor.matmul(
                        pf[:],
                        actsT[:, hs * t_own + tt * 128:
                              hs * t_own + (tt + 1) * 128],
                        w2_t[:, hs * C:(hs + 1) * C],
                        start=(hs == 0), stop=(hs == NHS - 1))
                xt = x_sb[:, tt * C:(tt + 1) * C]
                nc.vector.scalar_tensor_tensor(xt, pf[:], 1.0, xt,
                                               ALU.mult, ALU.add)
                nc.vector.tensor_add(xt, xt, b2_r)

        # ================= final LN -> AG -> lm_head =================
        wpool_cm.__exit__(None, None, None)
        lay_cm.__exit__(None, None, None)
        ps_bc_cm.__exit__(None, None, None)
        ps_o_cm.__exit__(None, None, None)
        ps_s_cm.__exit__(None, None, None)
        lm = pool(name="lm", bufs=1)
        ps_y = pool(name="ps_y", bufs=5, space="PSUM")
        lnf_t = lm.tile([128, 2 * C], F32, tag="lnf")
        nc.sync.dma_start(
            lnf_t[:].rearrange("p (k c) -> p k c", k=2), lnf_rep.ap()[:])
        ag_xf = ln_transpose_ag(lnf_t[:, 0:C], lnf_t[:, C:2 * C], "xf", lm)

        wout_t = lm.tile([128, CC * vsh], BF, tag="wout")
        nc.sync.dma_start(
            wout_t[:].rearrange("p (cc v) -> p cc v", cc=CC), wout_d.ap()[:])
        bout_t = lm.tile([128, vsh], F32, tag="bout")
        nc.sync.dma_start(bout_t[:], bout_rep.ap()[:])

        for r in range(n_cores):
            xtc = xtc_pool.tile([128, CC * t_own], BF, tag="xtc")
            nc.sync.dma_start(xtc[:], ag_xf[r])
            for st in range(n_tt):
                g_t0 = r * t_own + st * 128
                for vc in range(n_vc):
                    v0 = vc * vc_w
                    vw = min(vc_w, vsh - v0)
                    py = ps_y.tile([128, vc_w], F32, tag="ps_y")
                    for cc in range(CC):
                        nc.tensor.matmul(
                            py[:, 0:vw],
                            xtc[:, cc * t_own + st * 128:
                                cc * t_own + (st + 1) * 128],
                            wout_t[:, cc * vsh + v0: cc * vsh + v0 + vw],
                            start=(cc == 0), stop=(cc == CC - 1))
                    ysb = ysb_pool.tile([128, vc_w], F32, tag="ysb")
                    nc.vector.scalar_tensor_tensor(
                        ysb[:, 0:vw], py[:, 0:vw], 1.0,
                        bout_t[:, v0:v0 + vw], ALU.mult, ALU.add)
                    nc.sync.dma_start(
                        y.ap()[g_t0:g_t0 + 128, v0:v0 + vw], ysb[:, 0:vw])

        for cm in reversed(pools):
            cm.__exit__(None, None, None)

    nc.compile()
    return nc


# ======================================================================
# host side
# ======================================================================

def _pack_chunked(w):
    """[C_in, N] -> [128, C_in//128, N]: out[p, cc, n] = w[cc*128 + p, n]."""
    cin, n = w.shape
    return np.ascontiguousarray(w.reshape(cin // 128, 128, n).transpose(1, 0, 2))


def _shard_inputs(inputs, n_tt=4, vsh=4000, n_cores=NCORES):
    t_own = 128 * n_tt

    tok = np.asarray(inputs["input_tokens"])
    emb = np.asarray(inputs["tok_emb"], np.float32)
    x0_full = emb[tok]                                   # (B, Tb, C) fp32

    Wq = np.asarray(inputs["Wq"], np.float32)
    Wk = np.asarray(inputs["Wk"], np.float32)
    Wv = np.asarray(inputs["Wv"], np.float32)
    Wo = np.asarray(inputs["Wo"], np.float32)
    W1 = np.asarray(inputs["W1"], np.float32)
    W2 = np.asarray(inputs["W2"], np.float32)
    rpe = np.asarray(inputs["rpe"], np.float32)          # (L, NB, H)
    Wout = np.asarray(inputs["Wout"], np.float32)        # (C, V_tot)
    bout = np.asarray(inputs["bout"], np.float32)
    bo = np.asarray(inputs["bo"], np.float32)
    b1 = np.asarray(inputs["b1"], np.float32)
    b2 = np.asarray(inputs["b2"], np.float32)
    g1 = np.asarray(inputs["ln1_g"], np.float32)
    be1 = np.asarray(inputs["ln1_b"], np.float32)
    g2 = np.asarray(inputs["ln2_g"], np.float32)
    be2 = np.asarray(inputs["ln2_b"], np.float32)
    gf = np.asarray(inputs["lnf_g"], np.float32)
    bef = np.asarray(inputs["lnf_b"], np.float32)

    nL = Wq.shape[0]

    # shared (head/vocab-independent) packs
    wo_p = np.stack([_pack_chunked(Wo[l]) for l in range(nL)]).astype(BF16)
    w1_p = np.stack([_pack_chunked(W1[l]) for l in range(nL)]).astype(BF16)
    w2_p = np.stack([_pack_chunked(W2[l]) for l in range(nL)]).astype(BF16)
    b1t = np.ascontiguousarray(
        b1.reshape(nL, NHS, 128).transpose(0, 2, 1))     # [L,128,NHS]

    rep = np.empty((nL, 128, 6, C), np.float32)
    for l in range(nL):
        for i, vec in enumerate((g1[l], be1[l], g2[l], be2[l], bo[l], b2[l])):
            rep[l, :, i, :] = vec[None, :]
    lnf = np.empty((128, 2, C), np.float32)
    lnf[:, 0, :] = gf[None, :]
    lnf[:, 1, :] = bef[None, :]

    # mask+band tiles: mb[l, p, oi, j]; scores sT element (k=q0+oi*128+p,
    # q=q0+j): dqk = j - oi*128 - p
    p_i = np.arange(128)[:, None, None]
    oi_i = np.arange(4)[None, :, None]
    j_i = np.arange(512)[None, None, :]
    dqk = j_i - oi_i * 128 - p_i                         # (128, 4, 512)
    # spill tile: k-tile right before the chunk: k = q0-128+p, q = q0+j2
    p2 = np.arange(128)[:, None]
    j2 = np.arange(16)[None, :]
    dqk2 = j2 + 128 - p2                                 # (128, 16)

    in_maps = []
    for r in range(n_cores):
        h = r
        b_idx, blk = divmod(r, 4)
        x0 = np.ascontiguousarray(x0_full[b_idx, blk * t_own:(blk + 1) * t_own])

        wqkv = np.empty((nL, 128, 3, CC, D), np.float32)
        for l in range(nL):
            for k, W in enumerate((Wq, Wk, Wv)):
                sl = W[l][:, h * D:(h + 1) * D]          # (C, D)
                wqkv[l, :, k] = sl.reshape(CC, 128, D).transpose(1, 0, 2)

        mb = np.empty((nL, 128, 4, 512), np.float32)
        sp = np.empty((nL, 128, 16), np.float32)
        for l in range(nL):
            delta = rpe[l, :, h] - rpe[l, 0, h]          # (NB,)
            band_val = delta[np.clip(16 - dqk, 0, NB - 1)]
            mb[l] = np.where(dqk < 0, NEG,
                             np.where(dqk <= 16, band_val, 0.0))
            sp[l] = np.where((dqk2 >= 0) & (dqk2 <= 16),
                             delta[np.clip(16 - dqk2, 0, NB - 1)], 0.0)

        wout_sl = Wout[:, r * vsh:(r + 1) * vsh]
        wout_p = _pack_chunked(wout_sl).astype(BF16)
        bout_r = np.broadcast_to(bout[r * vsh:(r + 1) * vsh], (128, vsh))

        in_maps.append({
            "x0": x0,
            "wqkv": wqkv.astype(BF16),
            "wo": wo_p, "w1": w1_p, "w2": w2_p,
            "wout": wout_p,
            "b1t": b1t,
            "ln_rep": rep, "lnf_rep": lnf,
            "bout_rep": np.ascontiguousarray(bout_r, dtype=np.float32),
            "maskband": mb, "spill": sp,
        })
    return in_maps


_PROGRAM = None


def _assemble_output(per_core, inputs):
    """per_core: dict name -> [NCORES, ...] stacked per-core outputs."""
    Tb = inputs["input_tokens"].shape[1]
    vsh = V // NCORES
    out = np.empty((B, Tb, V), np.float32)
    for r in range(NCORES):
        yr = np.asarray(per_core["y"][r], np.float32)  # [B*Tb, vsh]
        out[:, :, r * vsh:(r + 1) * vsh] = yr.reshape(B, Tb, vsh)
    return out


def kernel(**inputs):
    global _PROGRAM
    if _PROGRAM is None:
        _PROGRAM = build_program()
    in_maps = _shard_inputs(inputs)
    res = run_bass_kernel_spmd(_PROGRAM, in_maps,
                               core_ids=list(range(NCORES)))
    per_core = {"y": [res.results[r]["y"] for r in range(NCORES)]}
    return _assemble_output(per_core, inputs)



# revision 14
# speedup vs baseline: 1.0712x; 1.0712x over previous
"""Trainium2 Bass kernel for a 4-layer GPT language model.

Model: B=2, T=2048, C=512, H=8 heads, L=4 layers, V=32000, relative-position
bias (33 buckets, clip +-16), causal attention, ReLU FFN (4C hidden),
final LN + untied output projection.

Sharding over 8 NeuronCores (one uniform SPMD program; all per-core
differences live in the input data):
 - attention: head-parallel (core r computes head r for all tokens/batches)
 - LN / residual / FFN / Wo projection: token-parallel (core r owns the 512
   contiguous tokens of batch r//4, block r%4)
 - lm_head: vocab-parallel (core r computes Wout columns [4000r, 4000(r+1)))

Pipelining (the point of this version):
 - per layer ONE AllGather of post-LN1 activations, split into two halves
   (token-tile pairs); each half is issued as soon as its pair finishes the
   FFN2+LN1' chain of the previous layer, hiding the ~15us collective
   latency behind the remaining FFN compute.  QKV consumption is split per
   pair so it can start on the first half.
 - attention chunk (b,qc) is emitted immediately after the QKV chunk that
   completes its k/v inputs -> no QKV/attention phase barrier.
 - q|k projections fused into one 128-wide stationary operand (Wq pre-scaled
   by 1/sqrt(D) on host); v stays token-major for the AV matmul.
 - all layer weights are prefetched one layer ahead on the gpsimd DMA path
   (double-buffered rings).
 - lm_head writes y in fp16 (halves the dominant HBM write traffic); bout
   is added on host during unsharding.

All matmul operands bf16 with fp32 PSUM accumulation; LN/softmax/residual in
fp32. Softmax runs without max-subtraction (scores are O(1) at this scale).
Causal mask + RPE bias are applied as one host-precomputed [128,512] add on
the pre-exp scores: -1e30 in the masked triangle, RPE delta vs rpe[l,0,h] in
the 17-wide diagonal band (softmax shift-invariance removes the constant).
The embedding gather, bf16 casts and layout packing happen on host; the host
reassembles the vocab-sharded per-core outputs and adds bout.
"""

import sys

for _p in ("/opt/trn_rl_repo", "/root/.axon_site/_ro/trn_rl_repo"):
    if _p not in sys.path:
        sys.path.append(_p)

import numpy as np
import ml_dtypes

import concourse.bass as bass
import concourse.bacc as bacc
import concourse.mybir as mybir
import concourse.tile as tile
from concourse import masks
from concourse.bass_utils import run_bass_kernel_spmd

BF16 = ml_dtypes.bfloat16
FP16 = np.float16

# model dims
B, C, H, L, V, MD = 2, 512, 8, 4, 32000, 16
D = C // H          # 64
HID = 4 * C         # 2048
NB = 2 * MD + 1     # 33
NCORES = 8
CC = C // 128       # 4 c-chunks
NHS = HID // 128    # 16 hidden slices
NEG = -1.0e30

F32 = mybir.dt.float32
BF = mybir.dt.bfloat16
F16 = mybir.dt.float16

T_OWN = 512          # tokens owned per core
PW = CC * 256        # AG payload cols per pair (cc-major, 256 tokens)


def build_program(n_cores=NCORES, vsh=4000, skip_collectives=False):
    t_own = T_OWN
    t_batch = 4 * t_own           # 2048
    t_glob = 2 * t_batch          # 4096
    n_qc = 4                      # 512-wide query chunks per batch
    n_kt = 16                     # 128-wide key tiles per batch
    n_vt = 32                     # v tiles (both batches)
    vc_w = 500
    n_vc = vsh // vc_w
    rg = [list(range(n_cores))]
    AF = mybir.ActivationFunctionType
    ALU = mybir.AluOpType

    nc = bacc.Bacc("TRN2", target_bir_lowering=False, debug=False,
                   num_devices=n_cores)

    # ---- per-core external inputs (host pre-packed, see _shard_inputs) ----
    x0 = nc.dram_tensor("x0", [t_own, C], F32, kind="ExternalInput")
    wqk_d = nc.dram_tensor("wqk", [L, 128, CC, 128], BF, kind="ExternalInput")
    wv_d = nc.dram_tensor("wv", [L, 128, CC, D], BF, kind="ExternalInput")
    wo_d = nc.dram_tensor("wo", [L, 128, CC, C], BF, kind="ExternalInput")
    w1_d = nc.dram_tensor("w1", [L, 128, CC, HID], BF, kind="ExternalInput")
    w2_d = nc.dram_tensor("w2", [L, 128, NHS, C], BF, kind="ExternalInput")
    wout_d = nc.dram_tensor("wout", [128, CC, vsh], BF, kind="ExternalInput")
    b1t_d = nc.dram_tensor("b1t", [L, 128, NHS], F32, kind="ExternalInput")
    # replicated per-column vectors: ln1_rep[l] = (g,b) of LN before attn of
    # layer l; slot L holds the final LN.  ln2_rep[l] = (g2, be2, bo, b2).
    ln1_rep = nc.dram_tensor("ln1_rep", [L + 1, 128, 2, C], BF,
                             kind="ExternalInput")
    ln2_rep = nc.dram_tensor("ln2_rep", [L, 128, 4, C], BF,
                             kind="ExternalInput")
    # combined causal-mask + RPE-delta tiles per diagonal offset oi
    mb_d = nc.dram_tensor("maskband", [L, 128, 4, 512], BF,
                          kind="ExternalInput")
    spill_d = nc.dram_tensor("spill", [L, 128, 16], BF, kind="ExternalInput")

    y = nc.dram_tensor("y", [t_glob, vsh], F16, kind="ExternalOutput")

    with tile.TileContext(nc) as tc:
        pools = []

        def pool(**kw):
            cm = tc.tile_pool(**kw)
            p = cm.__enter__()
            pools.append(cm)
            return p

        const = pool(name="const", bufs=1)
        persist = pool(name="persist", bufs=1)
        xtc_pool = pool(name="xtc", bufs=4)   # AG'd xT chunks [128, PW]
        small = pool(name="small", bufs=4)    # [128,1] LN scalars
        rows = pool(name="rows", bufs=2)      # [1,512] denom rows
        scratch = pool(name="scratch", bufs=2)
        ppool = pool(name="ppool", bufs=4)    # P (exp scores) tiles
        ysb_pool = pool(name="ysb", bufs=4)
        lay = pool(name="lay", bufs=1)
        wpool = pool(name="wpool", bufs=2)    # prefetched per-layer weights

        ps_t = pool(name="ps_t", bufs=1, space="PSUM")
        ps_proj = pool(name="ps_proj", bufs=2, space="PSUM")
        ps_s_cm = tc.tile_pool(name="ps_s", bufs=2, space="PSUM")
        ps_s = ps_s_cm.__enter__()
        ps_o_cm = tc.tile_pool(name="ps_o", bufs=2, space="PSUM")
        ps_o = ps_o_cm.__enter__()
        ps_bc_cm = tc.tile_pool(name="ps_bc", bufs=1, space="PSUM")
        ps_bc = ps_bc_cm.__enter__()

        dram = pool(name="dram", bufs=2, space="DRAM")

        # ---------------- constants ----------------
        ident = const.tile([128, 128], BF)
        masks.make_identity(nc, ident[:])
        ones_row = const.tile([1, 64], BF)
        nc.vector.memset(ones_row[:], 1.0)
        eps_t = const.tile([128, 1], F32)
        nc.vector.memset(eps_t[:], 1e-5)

        # residual stream, token-major fp32; tile tt at cols [tt*C,(tt+1)*C)
        x_sb = persist.tile([128, 4 * C], F32)
        nc.sync.dma_start(
            x_sb[:].rearrange("p (tt c) -> p tt c", tt=4),
            x0.ap().rearrange("(tt p) c -> p tt c", p=128))

        def layernorm(dst_ap, src_ap, g_ap, b_ap):
            """dst(bf16) = LN(src) with replicated gamma/beta; src [128,C]."""
            ssum = small.tile([128, 1], F32, tag="ln_ssum")
            nc.vector.tensor_reduce(ssum[:], src_ap, mybir.AxisListType.X,
                                    ALU.add)
            nmean = small.tile([128, 1], F32, tag="ln_nmean")
            nc.scalar.mul(nmean[:], ssum[:], -1.0 / C)
            xc = scratch.tile([128, C], F32, tag="ln_xc")
            nc.scalar.activation(xc[:], src_ap, AF.Identity, bias=nmean[:])
            sq = scratch.tile([128, C], F32, tag="ln_sq")
            vsum = small.tile([128, 1], F32, tag="ln_vsum")
            nc.scalar.activation(sq[:], xc[:], AF.Square, accum_out=vsum[:])
            std = small.tile([128, 1], F32, tag="ln_std")
            nc.scalar.activation(std[:], vsum[:], AF.Sqrt, bias=eps_t[:],
                                 scale=1.0 / C)
            rstd = small.tile([128, 1], F32, tag="ln_rstd")
            nc.vector.reciprocal(rstd[:], std[:])
            nc.vector.scalar_tensor_tensor(dst_ap, xc[:], rstd[:], g_ap,
                                           ALU.mult, ALU.mult)
            nc.vector.tensor_add(dst_ap, dst_ap, b_ap)

        def transpose_128(dst_ap, src_ap):
            """dst[128,128] = src[128,128].T via PE; bf16 in/out."""
            pt = ps_t.tile([128, 128], BF, tag="ps_t")
            nc.tensor.transpose(pt[:], src_ap, ident[:])
            nc.vector.tensor_copy(dst_ap, pt[:])

        # xT_own: pair-major [128, 2*PW]; pair p block: cc*256 + (tt%2)*128
        xT_own = lay.tile([128, 2 * PW], BF, tag="xT_own")

        def ln1_pair_to_ag(pair, g_ap, b_ap, tag):
            """LN the pair's two tiles -> xT_own pair block -> bounce -> AG.

            Returns the shared AG output tile [n_cores, 128, PW]."""
            for s in range(2):
                tt = 2 * pair + s
                xln = scratch.tile([128, C], BF, tag="ln_out")
                layernorm(xln[:], x_sb[:, tt * C:(tt + 1) * C], g_ap, b_ap)
                for cc in range(CC):
                    transpose_128(
                        xT_own[:, pair * PW + cc * 256 + s * 128:
                               pair * PW + cc * 256 + (s + 1) * 128],
                        xln[:, cc * 128:(cc + 1) * 128])
            bounce = dram.tile([128, PW], BF, tag=f"bnc_x{pair}")
            nc.sync.dma_start(bounce[:],
                              xT_own[:, pair * PW:(pair + 1) * PW])
            ag_out = dram.tile([n_cores, 128, PW], BF,
                               addr_space="Shared", tag=f"ag_x{tag}{pair}")
            if not skip_collectives:
                nc.gpsimd.collective_compute(
                    "AllGather", ALU.bypass, replica_groups=rg,
                    ins=[bounce[:].opt()], outs=[ag_out[:].opt()])
            return ag_out

        def load_layer_weights(l):
            """Prefetch all tensors needed by layer l (gpsimd DMA path).

            Returns dict of SBUF tiles.  With wpool bufs=2 this double-
            buffers against layer l-1's tiles still in use."""
            w = {}
            wqk = wpool.tile([128, CC * 128], BF, tag="wqk")
            nc.gpsimd.dma_start(
                wqk[:].rearrange("p (cc m) -> p cc m", cc=CC), wqk_d.ap()[l])
            wv = wpool.tile([128, CC * D], BF, tag="wv")
            nc.gpsimd.dma_start(
                wv[:].rearrange("p (cc d) -> p cc d", cc=CC), wv_d.ap()[l])
            mb = wpool.tile([128, 4 * 512], BF, tag="mb")
            nc.gpsimd.dma_start(
                mb[:].rearrange("p (oi j) -> p oi j", oi=4), mb_d.ap()[l])
            spill = wpool.tile([128, 16], BF, tag="spill")
            nc.gpsimd.dma_start(spill[:], spill_d.ap()[l])
            wo = wpool.tile([128, CC * C], BF, tag="wo")
            nc.gpsimd.dma_start(
                wo[:].rearrange("p (cc c) -> p cc c", cc=CC), wo_d.ap()[l])
            w1 = wpool.tile([128, CC * HID], BF, tag="w1")
            nc.gpsimd.dma_start(
                w1[:].rearrange("p (cc j) -> p cc j", cc=CC), w1_d.ap()[l])
            w2 = wpool.tile([128, NHS * C], BF, tag="w2")
            nc.gpsimd.dma_start(
                w2[:].rearrange("p (hs c) -> p hs c", hs=NHS), w2_d.ap()[l])
            b1t = wpool.tile([128, NHS], F32, tag="b1t")
            nc.gpsimd.dma_start(b1t[:], b1t_d.ap()[l])
            l2 = wpool.tile([128, 4 * C], BF, tag="l2rep")
            nc.gpsimd.dma_start(
                l2[:].rearrange("p (k c) -> p k c", k=4), ln2_rep.ap()[l])
            l1n = wpool.tile([128, 2 * C], BF, tag="l1rep")
            nc.gpsimd.dma_start(
                l1n[:].rearrange("p (k c) -> p k c", k=2),
                ln1_rep.ap()[l + 1])
            w.update(wqk=wqk, wv=wv, mb=mb, spill=spill, wo=wo, w1=w1,
                     w2=w2, b1t=b1t, l2=l2, l1n=l1n)
            return w

        # ---------------- prologue: LN1 of layer 0 + first AG ----------------
        l1_0 = const.tile([128, 2 * C], BF)
        nc.sync.dma_start(
            l1_0[:].rearrange("p (k c) -> p k c", k=2), ln1_rep.ap()[0])
        weights = load_layer_weights(0)
        ag_cur = [None, None]
        for p in range(2):
            ag_cur[p] = ln1_pair_to_ag(p, l1_0[:, 0:C], l1_0[:, C:2 * C],
                                       "l0_")

        # persistent attention tiles
        qf = lay.tile([64, t_glob], BF, tag="qf")
        kf = lay.tile([64, t_glob], BF, tag="kf")
        vaug = lay.tile([128, n_vt * 65], BF, tag="vaug")
        of = lay.tile([64, t_glob], BF, tag="of")
        o_sb = lay.tile([128, CC * t_own], BF, tag="o_sb")
        xT2 = lay.tile([128, CC * t_own], BF, tag="xT2")
        actsT = lay.tile([128, NHS * t_own], BF, tag="actsT")
        nc.vector.memset(
            vaug[:].rearrange("p (n e) -> p n e", e=65)[:, :, 64:65], 1.0)

        wout_t = [None, None]

        # ================= transformer layers =================
        for l in range(L):
            wqk, wv = weights["wqk"], weights["wv"]
            mb, spill = weights["mb"], weights["spill"]
            wo_t, w1_t, w2_t = weights["wo"], weights["w1"], weights["w2"]
            b1_t, l2rep, l1next = weights["b1t"], weights["l2"], weights["l1n"]
            g2, be2 = l2rep[:, 0:C], l2rep[:, C:2 * C]
            bo_r, b2_r = l2rep[:, 2 * C:3 * C], l2rep[:, 3 * C:4 * C]
            bounce_o = dram.tile([n_cores, 64, t_own], BF, tag="bnc_o")

            # ---- QKV (per AG half) interleaved with attention chunks ----
            for r in range(n_cores):
                cbase = r * t_own
                for p in range(2):
                    xtc = xtc_pool.tile([128, PW], BF, tag="xtc")
                    nc.sync.dma_start(xtc[:], ag_cur[p][r])
                    pq = ps_proj.tile([128, 256], F32, tag="mm")
                    for cc in range(CC):
                        nc.tensor.matmul(
                            pq[:], wqk[:, cc * 128:(cc + 1) * 128],
                            xtc[:, cc * 256:(cc + 1) * 256],
                            start=(cc == 0), stop=(cc == CC - 1))
                    cols = slice(cbase + p * 256, cbase + (p + 1) * 256)
                    nc.scalar.copy(qf[:, cols], pq[0:64, :])
                    nc.vector.tensor_copy(kf[:, cols], pq[64:128, :])
                    for s in range(2):
                        pv = ps_proj.tile([128, D], F32, tag="mm")
                        for cc in range(CC):
                            nc.tensor.matmul(
                                pv[:],
                                xtc[:, cc * 256 + s * 128:
                                    cc * 256 + (s + 1) * 128],
                                wv[:, cc * D:(cc + 1) * D],
                                start=(cc == 0), stop=(cc == CC - 1))
                        vt = r * 4 + p * 2 + s
                        nc.vector.tensor_copy(
                            vaug[:, vt * 65: vt * 65 + 64], pv[:])

                # ---- attention chunk (b, qc) = (r//4, r%4) now ready ----
                b, qc = divmod(r, 4)
                q0 = qc * 512
                qcols = slice(b * t_batch + q0, b * t_batch + q0 + 512)
                po = ps_o.tile([65, 512], F32, tag="ps_o")
                nkt = 4 * (qc + 1)
                for kt in range(nkt):
                    k0 = kt * 128
                    ps = ps_s.tile([128, 512], F32, tag="ps_s")
                    nc.tensor.matmul(
                        ps[:],
                        kf[:, b * t_batch + k0: b * t_batch + k0 + 128],
                        qf[:, qcols], start=True, stop=True)
                    if k0 >= q0:      # diagonal region: mask + RPE band
                        oi = (k0 - q0) // 128
                        nc.vector.tensor_add(
                            ps[:], ps[:], mb[:, oi * 512:(oi + 1) * 512])
                    elif k0 == q0 - 128:  # band spill from prev chunk
                        nc.vector.tensor_add(ps[:, 0:16], ps[:, 0:16],
                                             spill[:])
                    p_t = ppool.tile([128, 512], BF, tag="p")
                    nc.scalar.activation(p_t[:], ps[:], AF.Exp)
                    nc.tensor.matmul(
                        po[:],
                        vaug[:, (b * n_kt + kt) * 65:
                             (b * n_kt + kt) * 65 + 65],
                        p_t[:], start=(kt == 0), stop=(kt == nkt - 1))
                # normalize: 1/denom row broadcast via K=1 matmul
                den = rows.tile([1, 512], F32, tag="den")
                nc.vector.tensor_copy(den[:], po[64:65, :])
                rden = rows.tile([1, 512], F32, tag="rden")
                nc.vector.reciprocal(rden[:], den[:])
                rb = rows.tile([1, 512], BF, tag="rb")
                nc.vector.tensor_copy(rb[:], rden[:])
                pbc = ps_bc.tile([64, 512], F32, tag="ps_bc")
                nc.tensor.matmul(pbc[:], ones_row[:], rb[:],
                                 start=True, stop=True)
                bc_sb = scratch.tile([64, 512], F32, tag="bc_sb")
                nc.scalar.copy(bc_sb[:], pbc[:])
                nc.vector.tensor_mul(of[:, qcols], po[0:64, :], bc_sb[:])
                # this chunk IS destination core r's payload: stage it now
                nc.sync.dma_start(bounce_o[r], of[:, qcols])

            # ---- AllToAll: deliver all heads' o for own tokens ----
            o_own = dram.tile([n_cores * 64, t_own], BF, tag="a2a_o")
            if not skip_collectives:
                nc.gpsimd.collective_compute(
                    "AllToAll", ALU.bypass, replica_groups=rg,
                    ins=[bounce_o[:].opt()], outs=[o_own[:].opt()])

            # prefetch next layer's weights while the A2A is in flight
            if l + 1 < L:
                weights = load_layer_weights(l + 1)
            else:
                # last layer: prefetch the lm_head weight halves instead,
                # into the idle w1/w2 ring slots
                for h in range(2):
                    wt = wpool.tile([128, CC * (vsh // 2)], BF,
                                    tag=("w1", "w2")[h])
                    nc.gpsimd.dma_start(
                        wt[:].rearrange("p (cc v) -> p cc v", cc=CC),
                        wout_d.ap()[:, :, h * (vsh // 2):
                                    (h + 1) * (vsh // 2)])
                    wout_t[h] = wt

            # o feature-major in SBUF: o_sb[:, cc*t_own + t]
            nc.sync.dma_start(
                o_sb[:].rearrange("p (cc t) -> p cc t", cc=CC),
                o_own[:].rearrange("(cc p) t -> p cc t", p=128))

            # ---- Wo projection + residual (all 4 tiles) ----
            for tt in range(4):
                px = ps_proj.tile([128, C], F32, tag="mm")
                for cc in range(CC):
                    nc.tensor.matmul(
                        px[:],
                        o_sb[:, cc * t_own + tt * 128:
                             cc * t_own + (tt + 1) * 128],
                        wo_t[:, cc * C:(cc + 1) * C],
                        start=(cc == 0), stop=(cc == CC - 1))
                xt = x_sb[:, tt * C:(tt + 1) * C]
                nc.vector.scalar_tensor_tensor(xt, px[:], 1.0, xt,
                                               ALU.mult, ALU.add)
                nc.vector.tensor_add(xt, xt, bo_r)
                # LN2 immediately per tile (ACT/DVE; overlaps next Wo MMs)
                xln = scratch.tile([128, C], BF, tag="ln_out")
                layernorm(xln[:], xt, g2, be2)
                for cc in range(CC):
                    transpose_128(
                        xT2[:, cc * t_own + tt * 128:
                            cc * t_own + (tt + 1) * 128],
                        xln[:, cc * 128:(cc + 1) * 128])

            # ---- FFN + next-layer LN1 + AG, per tile pair ----
            if l + 1 < L:
                g1n, be1n = l1next[:, 0:C], l1next[:, C:2 * C]
            else:
                lnf_t = const.tile([128, 2 * C], BF, tag="lnf")
                nc.sync.dma_start(
                    lnf_t[:].rearrange("p (k c) -> p k c", k=2),
                    ln1_rep.ap()[L])
                g1n, be1n = lnf_t[:, 0:C], lnf_t[:, C:2 * C]

            ag_nxt = [None, None]
            for pair in range(2):
                # FFN1 for this pair (N=256 streams)
                for hs in range(NHS):
                    ph = ps_proj.tile([128, 256], F32, tag="mm")
                    for cc in range(CC):
                        nc.tensor.matmul(
                            ph[:],
                            w1_t[:, cc * HID + hs * 128:
                                 cc * HID + (hs + 1) * 128],
                            xT2[:, cc * t_own + pair * 256:
                                cc * t_own + (pair + 1) * 256],
                            start=(cc == 0), stop=(cc == CC - 1))
                    nc.scalar.activation(
                        actsT[:, hs * t_own + pair * 256:
                              hs * t_own + (pair + 1) * 256],
                        ph[:], AF.Relu, bias=b1_t[:, hs:hs + 1])
                # FFN2 + residual per tile of the pair
                for s in range(2):
                    tt = 2 * pair + s
                    pf = ps_proj.tile([128, C], F32, tag="mm")
                    for hs in range(NHS):
                        nc.tensor.matmul(
                            pf[:],
                            actsT[:, hs * t_own + tt * 128:
                                  hs * t_own + (tt + 1) * 128],
                            w2_t[:, hs * C:(hs + 1) * C],
                            start=(hs == 0), stop=(hs == NHS - 1))
                    xt = x_sb[:, tt * C:(tt + 1) * C]
                    nc.vector.scalar_tensor_tensor(xt, pf[:], 1.0, xt,
                                                   ALU.mult, ALU.add)
                    nc.vector.tensor_add(xt, xt, b2_r)
                # LN1 of next layer (or final LN) + AG for this pair
                ag_nxt[pair] = ln1_pair_to_ag(pair, g1n, be1n, f"l{l + 1}_")
            ag_cur = ag_nxt

        # ================= lm_head (vocab-sharded) =================
        ps_bc_cm.__exit__(None, None, None)
        ps_o_cm.__exit__(None, None, None)
        ps_s_cm.__exit__(None, None, None)
        ps_y = pool(name="ps_y", bufs=5, space="PSUM")

        for p in range(2):
            for r in range(n_cores):
                xtc = xtc_pool.tile([128, PW], BF, tag="xtc")
                nc.sync.dma_start(xtc[:], ag_cur[p][r])
                for s in range(2):
                    st = 2 * p + s
                    g_t0 = r * t_own + st * 128
                    for vc in range(n_vc):
                        v0 = vc * vc_w
                        h = v0 // (vsh // 2)
                        hv0 = v0 - h * (vsh // 2)
                        hw = vsh // 2
                        py = ps_y.tile([128, vc_w], F32, tag="ps_y")
                        for cc in range(CC):
                            nc.tensor.matmul(
                                py[:],
                                xtc[:, cc * 256 + s * 128:
                                    cc * 256 + (s + 1) * 128],
                                wout_t[h][:, cc * hw + hv0:
                                          cc * hw + hv0 + vc_w],
                                start=(cc == 0), stop=(cc == CC - 1))
                        ysb = ysb_pool.tile([128, vc_w], F16, tag="ysb")
                        if vc % 2 == 0:
                            nc.vector.tensor_copy(ysb[:], py[:])
                        else:
                            nc.scalar.copy(ysb[:], py[:])
                        nc.sync.dma_start(
                            y.ap()[g_t0:g_t0 + 128, v0:v0 + vc_w], ysb[:])

        for cm in reversed(pools):
            cm.__exit__(None, None, None)

    nc.compile()
    return nc


# ======================================================================
# host side
# ======================================================================

def _pack_chunked(w):
    """[C_in, N] -> [128, C_in//128, N]: out[p, cc, n] = w[cc*128 + p, n]."""
    cin, n = w.shape
    return np.ascontiguousarray(w.reshape(cin // 128, 128, n).transpose(1, 0, 2))


def _shard_inputs(inputs, vsh=4000, n_cores=NCORES):
    t_own = T_OWN

    tok = np.asarray(inputs["input_tokens"])
    emb = np.asarray(inputs["tok_emb"], np.float32)
    x0_full = emb[tok]                                   # (B, Tb, C) fp32

    Wq = np.asarray(inputs["Wq"], np.float32)
    Wk = np.asarray(inputs["Wk"], np.float32)
    Wv = np.asarray(inputs["Wv"], np.float32)
    Wo = np.asarray(inputs["Wo"], np.float32)
    W1 = np.asarray(inputs["W1"], np.float32)
    W2 = np.asarray(inputs["W2"], np.float32)
    rpe = np.asarray(inputs["rpe"], np.float32)          # (L, NB, H)
    Wout = np.asarray(inputs["Wout"], np.float32)        # (C, V_tot)
    bo = np.asarray(inputs["bo"], np.float32)
    b1 = np.asarray(inputs["b1"], np.float32)
    b2 = np.asarray(inputs["b2"], np.float32)
    g1 = np.asarray(inputs["ln1_g"], np.float32)
    be1 = np.asarray(inputs["ln1_b"], np.float32)
    g2 = np.asarray(inputs["ln2_g"], np.float32)
    be2 = np.asarray(inputs["ln2_b"], np.float32)
    gf = np.asarray(inputs["lnf_g"], np.float32)
    bef = np.asarray(inputs["lnf_b"], np.float32)

    nL = Wq.shape[0]

    # shared (head/vocab-independent) packs
    wo_p = np.stack([_pack_chunked(Wo[l]) for l in range(nL)]).astype(BF16)
    w1_p = np.stack([_pack_chunked(W1[l]) for l in range(nL)]).astype(BF16)
    w2_p = np.stack([_pack_chunked(W2[l]) for l in range(nL)]).astype(BF16)
    b1t = np.ascontiguousarray(
        b1.reshape(nL, NHS, 128).transpose(0, 2, 1))     # [L,128,NHS]

    rep1 = np.empty((nL + 1, 128, 2, C), np.float32)  # cast to bf16 below
    for l in range(nL):
        rep1[l, :, 0, :] = g1[l][None, :]
        rep1[l, :, 1, :] = be1[l][None, :]
    rep1[nL, :, 0, :] = gf[None, :]
    rep1[nL, :, 1, :] = bef[None, :]
    rep2 = np.empty((nL, 128, 4, C), np.float32)
    for l in range(nL):
        for i, vec in enumerate((g2[l], be2[l], bo[l], b2[l])):
            rep2[l, :, i, :] = vec[None, :]

    # mask+band tiles: mb[l, p, oi, j]; scores sT element (k=q0+oi*128+p,
    # q=q0+j): dqk = j - oi*128 - p
    p_i = np.arange(128)[:, None, None]
    oi_i = np.arange(4)[None, :, None]
    j_i = np.arange(512)[None, None, :]
    dqk = j_i - oi_i * 128 - p_i                         # (128, 4, 512)
    # spill tile: k-tile right before the chunk: k = q0-128+p, q = q0+j2
    p2 = np.arange(128)[:, None]
    j2 = np.arange(16)[None, :]
    dqk2 = j2 + 128 - p2                                 # (128, 16)

    scale = 1.0 / np.sqrt(D)
    in_maps = []
    for r in range(n_cores):
        h = r
        b_idx, blk = divmod(r, 4)
        x0 = np.ascontiguousarray(x0_full[b_idx, blk * t_own:(blk + 1) * t_own])

        # fused q|k stationary [L, 128, CC, 128]; Wq pre-scaled by 1/sqrt(D)
        wqk = np.empty((nL, 128, CC, 128), np.float32)
        wvp = np.empty((nL, 128, CC, D), np.float32)
        for l in range(nL):
            q_sl = Wq[l][:, h * D:(h + 1) * D] * scale   # (C, D)
            k_sl = Wk[l][:, h * D:(h + 1) * D]
            v_sl = Wv[l][:, h * D:(h + 1) * D]
            wqk[l, :, :, 0:64] = q_sl.reshape(CC, 128, D).transpose(1, 0, 2)
            wqk[l, :, :, 64:128] = k_sl.reshape(CC, 128, D).transpose(1, 0, 2)
            wvp[l] = v_sl.reshape(CC, 128, D).transpose(1, 0, 2)

        mb = np.empty((nL, 128, 4, 512), np.float32)
        sp = np.empty((nL, 128, 16), np.float32)
        for l in range(nL):
            delta = rpe[l, :, h] - rpe[l, 0, h]          # (NB,)
            band_val = delta[np.clip(16 - dqk, 0, NB - 1)]
            mb[l] = np.where(dqk < 0, NEG,
                             np.where(dqk <= 16, band_val, 0.0))
            sp[l] = np.where((dqk2 >= 0) & (dqk2 <= 16),
                             delta[np.clip(16 - dqk2, 0, NB - 1)], 0.0)

        wout_sl = Wout[:, r * vsh:(r + 1) * vsh]
        wout_p = _pack_chunked(wout_sl).astype(BF16)

        in_maps.append({
            "x0": x0,
            "wqk": wqk.astype(BF16), "wv": wvp.astype(BF16),
            "wo": wo_p, "w1": w1_p, "w2": w2_p,
            "wout": wout_p,
            "b1t": b1t,
            "ln1_rep": rep1.astype(BF16), "ln2_rep": rep2.astype(BF16),
            "maskband": mb.astype(BF16), "spill": sp.astype(BF16),
        })
    return in_maps


_PROGRAM = None


def _assemble_output(per_core, inputs):
    """per_core: dict name -> [NCORES, ...] stacked per-core outputs."""
    Tb = inputs["input_tokens"].shape[1]
    vsh = V // NCORES
    bout = np.asarray(inputs["bout"], np.float32)
    out = np.empty((B, Tb, V), np.float32)
    for r in range(NCORES):
        yr = np.asarray(per_core["y"][r]).astype(np.float32)  # [B*Tb, vsh]
        yr += bout[r * vsh:(r + 1) * vsh]
        out[:, :, r * vsh:(r + 1) * vsh] = yr.reshape(B, Tb, vsh)
    return out


def kernel(**inputs):
    global _PROGRAM
    if _PROGRAM is None:
        _PROGRAM = build_program()
    in_maps = _shard_inputs(inputs)
    res = run_bass_kernel_spmd(_PROGRAM, in_maps,
                               core_ids=list(range(NCORES)))
    per_core = {"y": [res.results[r]["y"] for r in range(NCORES)]}
    return _assemble_output(per_core, inputs)


# revision 18
# speedup vs baseline: 1.2470x; 1.1641x over previous
"""Trainium2 Bass kernel for a 4-layer GPT language model.

Model: B=2, T=2048, C=512, H=8 heads, L=4 layers, V=32000, relative-position
bias (33 buckets, clip +-16), causal attention, ReLU FFN (4C hidden),
final LN + untied output projection.

Sharding over 8 NeuronCores (one uniform SPMD program; all per-core
differences live in the input data):
 - attention: head-parallel (core r computes head r for all tokens/batches)
 - LN / residual / FFN / Wo projection: token-parallel (core r owns the 512
   contiguous tokens of batch r//4, block r%4)
 - lm_head: vocab-parallel (core r computes Wout columns [4000r, 4000(r+1)))

Pipelining (the point of this version):
 - per layer ONE AllGather of post-LN1 activations, split into two halves
   (token-tile pairs); each half is issued as soon as its pair finishes the
   FFN2+LN1' chain of the previous layer, hiding the ~15us collective
   latency behind the remaining FFN compute.  QKV consumption is split per
   pair so it can start on the first half.
 - attention chunk (b,qc) is emitted immediately after the QKV chunk that
   completes its k/v inputs -> no QKV/attention phase barrier.
 - q|k projections fused into one 128-wide stationary operand (Wq pre-scaled
   by 1/sqrt(D) on host); v stays token-major for the AV matmul.
 - all layer weights are prefetched one layer ahead on the gpsimd DMA path
   (double-buffered rings).
 - lm_head writes y in fp16 (halves the dominant HBM write traffic); bout
   is added on host during unsharding.

All matmul operands bf16 with fp32 PSUM accumulation; LN/softmax/residual in
fp32. Softmax runs without max-subtraction (scores are O(1) at this scale).
Causal mask + RPE bias are applied as one host-precomputed [128,512] add on
the pre-exp scores: -1e30 in the masked triangle, RPE delta vs rpe[l,0,h] in
the 17-wide diagonal band (softmax shift-invariance removes the constant).
The embedding gather, bf16 casts and layout packing happen on host; the host
reassembles the vocab-sharded per-core outputs and adds bout.
"""

import sys

for _p in ("/opt/trn_rl_repo", "/root/.axon_site/_ro/trn_rl_repo"):
    if _p not in sys.path:
        sys.path.append(_p)

import numpy as np
import ml_dtypes

import concourse.bass as bass
import concourse.bacc as bacc
import concourse.mybir as mybir
import concourse.tile as tile
from concourse import masks
from concourse.bass_utils import run_bass_kernel_spmd

BF16 = ml_dtypes.bfloat16
FP16 = np.float16

# model dims
B, C, H, L, V, MD = 2, 512, 8, 4, 32000, 16
D = C // H          # 64
HID = 4 * C         # 2048
NB = 2 * MD + 1     # 33
NCORES = 8
CC = C // 128       # 4 c-chunks
NHS = HID // 128    # 16 hidden slices
NEG = -1.0e30

F32 = mybir.dt.float32
BF = mybir.dt.bfloat16
F16 = mybir.dt.float16

T_OWN = 512          # tokens owned per core
PW = CC * 256        # AG payload cols per pair (cc-major, 256 tokens)


def build_program(n_cores=NCORES, vsh=4000, skip_collectives=False):
    t_own = T_OWN
    t_batch = 4 * t_own           # 2048
    t_glob = 2 * t_batch          # 4096
    n_qc = 4                      # 512-wide query chunks per batch
    n_kt = 16                     # 128-wide key tiles per batch
    n_vt = 32                     # v tiles (both batches)
    vc_w = 500
    n_vc = vsh // vc_w
    rg = [list(range(n_cores))]
    AF = mybir.ActivationFunctionType
    ALU = mybir.AluOpType

    nc = bacc.Bacc("TRN2", target_bir_lowering=False, debug=False,
                   num_devices=n_cores)

    # ---- per-core external inputs (host pre-packed, see _shard_inputs) ----
    x0 = nc.dram_tensor("x0", [t_own, C], F32, kind="ExternalInput")
    wqk_d = nc.dram_tensor("wqk", [L, 128, CC, 128], BF, kind="ExternalInput")
    wv_d = nc.dram_tensor("wv", [L, 128, CC, D], BF, kind="ExternalInput")
    wo_d = nc.dram_tensor("wo", [L, 128, CC, C], BF, kind="ExternalInput")
    w1_d = nc.dram_tensor("w1", [L, 128, CC, HID], BF, kind="ExternalInput")
    w2_d = nc.dram_tensor("w2", [L, 128, NHS, C], BF, kind="ExternalInput")
    wout_d = nc.dram_tensor("wout", [128, CC, vsh], BF, kind="ExternalInput")
    b1t_d = nc.dram_tensor("b1t", [L, 128, NHS], F32, kind="ExternalInput")
    # replicated per-column vectors: ln1_rep[l] = (g,b) of LN before attn of
    # layer l; slot L holds the final LN.  ln2_rep[l] = (g2, be2, bo, b2).
    ln1_rep = nc.dram_tensor("ln1_rep", [L + 1, 128, 2, C], BF,
                             kind="ExternalInput")
    ln2_rep = nc.dram_tensor("ln2_rep", [L, 128, 4, C], BF,
                             kind="ExternalInput")
    # combined causal-mask + RPE-delta tiles per diagonal offset oi
    mb_d = nc.dram_tensor("maskband", [L, 128, 4, 512], BF,
                          kind="ExternalInput")
    spill_d = nc.dram_tensor("spill", [L, 128, 16], BF, kind="ExternalInput")

    y = nc.dram_tensor("y", [t_glob, vsh], F16, kind="ExternalOutput")

    with tile.TileContext(nc) as tc:
        pools = []

        def pool(**kw):
            cm = tc.tile_pool(**kw)
            p = cm.__enter__()
            pools.append(cm)
            return p

        const = pool(name="const", bufs=1)
        persist = pool(name="persist", bufs=1)
        xtc_pool = pool(name="xtc", bufs=4)   # AG'd xT chunks [128, PW]
        small = pool(name="small", bufs=4)    # [128,1] LN scalars
        rows = pool(name="rows", bufs=2)      # [1,512] denom rows
        scratch = pool(name="scratch", bufs=2)
        ppool = pool(name="ppool", bufs=4)    # P (exp scores) tiles
        ysb_pool = pool(name="ysb", bufs=4)
        lay = pool(name="lay", bufs=1)
        wpool = pool(name="wpool", bufs=2)    # prefetched per-layer weights

        ps_t = pool(name="ps_t", bufs=1, space="PSUM")
        ps_proj = pool(name="ps_proj", bufs=2, space="PSUM")
        ps_s_cm = tc.tile_pool(name="ps_s", bufs=2, space="PSUM")
        ps_s = ps_s_cm.__enter__()
        ps_o_cm = tc.tile_pool(name="ps_o", bufs=2, space="PSUM")
        ps_o = ps_o_cm.__enter__()
        ps_bc_cm = tc.tile_pool(name="ps_bc", bufs=1, space="PSUM")
        ps_bc = ps_bc_cm.__enter__()

        dram = pool(name="dram", bufs=2, space="DRAM")

        # ---------------- constants ----------------
        ident = const.tile([128, 128], BF)
        masks.make_identity(nc, ident[:])
        ones_row = const.tile([1, 64], BF)
        nc.vector.memset(ones_row[:], 1.0)
        eps_t = const.tile([128, 1], F32)
        nc.vector.memset(eps_t[:], 1e-5)

        # residual stream, token-major fp32; tile tt at cols [tt*C,(tt+1)*C)
        x_sb = persist.tile([128, 4 * C], F32)
        for tt in range(4):
            nc.sync.dma_start(
                x_sb[:, tt * C:(tt + 1) * C],
                x0.ap().rearrange("(tt p) c -> p tt c", p=128)[:, tt])

        def layernorm(dst_ap, src_ap, g_ap, b_ap):
            """dst(bf16) = LN(src) with replicated gamma/beta; src [128,C]."""
            ssum = small.tile([128, 1], F32, tag="ln_ssum")
            nc.vector.tensor_reduce(ssum[:], src_ap, mybir.AxisListType.X,
                                    ALU.add)
            nmean = small.tile([128, 1], F32, tag="ln_nmean")
            nc.scalar.mul(nmean[:], ssum[:], -1.0 / C)
            xc = scratch.tile([128, C], F32, tag="ln_xc")
            nc.scalar.activation(xc[:], src_ap, AF.Identity, bias=nmean[:])
            sq = scratch.tile([128, C], F32, tag="ln_sq")
            vsum = small.tile([128, 1], F32, tag="ln_vsum")
            nc.scalar.activation(sq[:], xc[:], AF.Square, accum_out=vsum[:])
            std = small.tile([128, 1], F32, tag="ln_std")
            nc.scalar.activation(std[:], vsum[:], AF.Sqrt, bias=eps_t[:],
                                 scale=1.0 / C)
            rstd = small.tile([128, 1], F32, tag="ln_rstd")
            nc.vector.reciprocal(rstd[:], std[:])
            nc.vector.scalar_tensor_tensor(dst_ap, xc[:], rstd[:], g_ap,
                                           ALU.mult, ALU.mult)
            nc.vector.tensor_add(dst_ap, dst_ap, b_ap)

        def transpose_128(dst_ap, src_ap):
            """dst[128,128] = src[128,128].T via PE; bf16 in/out."""
            pt = ps_t.tile([128, 128], BF, tag="ps_t")
            nc.tensor.transpose(pt[:], src_ap, ident[:])
            nc.vector.tensor_copy(dst_ap, pt[:])

        # xT_own: pair-major [128, 2*PW]; pair p block: cc*256 + (tt%2)*128
        xT_own = lay.tile([128, 2 * PW], BF, tag="xT_own")

        def ln1_pair(pair, g_ap, b_ap):
            """LN the pair's two tiles (ACT/DVE only); returns bf16 tiles."""
            xlns = []
            for s in range(2):
                tt = 2 * pair + s
                xln = scratch.tile([128, C], BF, tag=f"ln_out{s}")
                layernorm(xln[:], x_sb[:, tt * C:(tt + 1) * C], g_ap, b_ap)
                xlns.append(xln)
            return xlns

        def transpose_pair_ag(pair, xlns, tag):
            """Transpose the pair into xT_own, bounce to DRAM, AllGather.

            Returns the shared AG output tile [n_cores, 128, PW]."""
            for s in range(2):
                for cc in range(CC):
                    transpose_128(
                        xT_own[:, pair * PW + cc * 256 + s * 128:
                               pair * PW + cc * 256 + (s + 1) * 128],
                        xlns[s][:, cc * 128:(cc + 1) * 128])
            bounce = dram.tile([128, PW], BF, tag=f"bnc_x{pair}")
            nc.sync.dma_start(bounce[:],
                              xT_own[:, pair * PW:(pair + 1) * PW])
            ag_out = dram.tile([n_cores, 128, PW], BF,
                               addr_space="Shared", tag=f"ag_x{tag}{pair}")
            if not skip_collectives:
                nc.gpsimd.collective_compute(
                    "AllGather", ALU.bypass, replica_groups=rg,
                    ins=[bounce[:].opt()], outs=[ag_out[:].opt()])
            return ag_out

        def ln1_pair_to_ag(pair, g_ap, b_ap, tag):
            return transpose_pair_ag(pair, ln1_pair(pair, g_ap, b_ap), tag)

        def load_layer_weights(l):
            """Prefetch all tensors needed by layer l (gpsimd DMA path).

            Returns dict of SBUF tiles.  With wpool bufs=2 this double-
            buffers against layer l-1's tiles still in use."""
            w = {}
            wqk = wpool.tile([128, CC * 128], BF, tag="wqk")
            nc.gpsimd.dma_start(
                wqk[:].rearrange("p (cc m) -> p cc m", cc=CC), wqk_d.ap()[l])
            wv = wpool.tile([128, CC * D], BF, tag="wv")
            nc.gpsimd.dma_start(
                wv[:].rearrange("p (cc d) -> p cc d", cc=CC), wv_d.ap()[l])
            mb = wpool.tile([128, 4 * 512], BF, tag="mb")
            nc.gpsimd.dma_start(
                mb[:].rearrange("p (oi j) -> p oi j", oi=4), mb_d.ap()[l])
            spill = wpool.tile([128, 16], BF, tag="spill")
            nc.gpsimd.dma_start(spill[:], spill_d.ap()[l])
            wo = wpool.tile([128, CC * C], BF, tag="wo")
            nc.gpsimd.dma_start(
                wo[:].rearrange("p (cc c) -> p cc c", cc=CC), wo_d.ap()[l])
            w1 = wpool.tile([128, CC * HID], BF, tag="w1")
            nc.gpsimd.dma_start(
                w1[:].rearrange("p (cc j) -> p cc j", cc=CC), w1_d.ap()[l])
            w2 = wpool.tile([128, NHS * C], BF, tag="w2")
            nc.gpsimd.dma_start(
                w2[:].rearrange("p (hs c) -> p hs c", hs=NHS), w2_d.ap()[l])
            b1t = wpool.tile([128, NHS], F32, tag="b1t")
            nc.gpsimd.dma_start(b1t[:], b1t_d.ap()[l])
            l2 = wpool.tile([128, 4 * C], BF, tag="l2rep")
            nc.gpsimd.dma_start(
                l2[:].rearrange("p (k c) -> p k c", k=4), ln2_rep.ap()[l])
            l1n = wpool.tile([128, 2 * C], BF, tag="l1rep")
            nc.gpsimd.dma_start(
                l1n[:].rearrange("p (k c) -> p k c", k=2),
                ln1_rep.ap()[l + 1])
            w.update(wqk=wqk, wv=wv, mb=mb, spill=spill, wo=wo, w1=w1,
                     w2=w2, b1t=b1t, l2=l2, l1n=l1n)
            return w

        # ---------------- prologue: LN1 of layer 0 + first AG ----------------
        l1_0 = const.tile([128, 2 * C], BF)
        nc.sync.dma_start(
            l1_0[:].rearrange("p (k c) -> p k c", k=2), ln1_rep.ap()[0])
        weights = load_layer_weights(0)
        ag_cur = [None, None]
        for p in range(2):
            ag_cur[p] = ln1_pair_to_ag(p, l1_0[:, 0:C], l1_0[:, C:2 * C],
                                       "l0_")

        # persistent attention tiles
        qf = lay.tile([64, t_glob], BF, tag="qf")
        kf = lay.tile([64, t_glob], BF, tag="kf")
        vaug = lay.tile([128, n_vt * 65], BF, tag="vaug")
        of = lay.tile([64, t_glob], BF, tag="of")
        o_sb = lay.tile([128, CC * t_own], BF, tag="o_sb")
        xT2 = lay.tile([128, CC * t_own], BF, tag="xT2")
        actsT = lay.tile([128, NHS * t_own], BF, tag="actsT")
        nc.vector.memset(
            vaug[:].rearrange("p (n e) -> p n e", e=65)[:, :, 64:65], 1.0)

        wout_t = [None, None]

        # ================= transformer layers =================
        for l in range(L):
            wqk, wv = weights["wqk"], weights["wv"]
            mb, spill = weights["mb"], weights["spill"]
            wo_t, w1_t, w2_t = weights["wo"], weights["w1"], weights["w2"]
            b1_t, l2rep, l1next = weights["b1t"], weights["l2"], weights["l1n"]
            g2, be2 = l2rep[:, 0:C], l2rep[:, C:2 * C]
            bo_r, b2_r = l2rep[:, 2 * C:3 * C], l2rep[:, 3 * C:4 * C]
            bounce_o = dram.tile([n_cores, 64, t_own], BF, tag="bnc_o")

            # ---- QKV pair-major (pair 0 for all chunks covers AG_1
            # latency), attention chunks interleaved in the pair-1 sweep ----
            def qkv_half(r, p):
                cbase = r * t_own
                xtc = xtc_pool.tile([128, PW], BF, tag="xtc")
                nc.sync.dma_start(xtc[:], ag_cur[p][r])
                pq = ps_proj.tile([128, 256], F32, tag="mm")
                for cc in range(CC):
                    nc.tensor.matmul(
                        pq[:], wqk[:, cc * 128:(cc + 1) * 128],
                        xtc[:, cc * 256:(cc + 1) * 256],
                        start=(cc == 0), stop=(cc == CC - 1))
                cols = slice(cbase + p * 256, cbase + (p + 1) * 256)
                nc.vector.tensor_copy(qf[:, cols], pq[0:64, :])
                nc.vector.tensor_copy(kf[:, cols], pq[64:128, :])
                for s in range(2):
                    pv = ps_proj.tile([128, D], F32, tag="mm")
                    for cc in range(CC):
                        nc.tensor.matmul(
                            pv[:],
                            xtc[:, cc * 256 + s * 128:
                                cc * 256 + (s + 1) * 128],
                            wv[:, cc * D:(cc + 1) * D],
                            start=(cc == 0), stop=(cc == CC - 1))
                    vt = r * 4 + p * 2 + s
                    nc.vector.tensor_copy(
                        vaug[:, vt * 65: vt * 65 + 64], pv[:])

            for r in range(n_cores):
                qkv_half(r, 0)
            for r in range(n_cores):
                qkv_half(r, 1)

                # ---- attention chunk (b, qc) = (r//4, r%4) now ready ----
                b, qc = divmod(r, 4)
                q0 = qc * 512
                qcols = slice(b * t_batch + q0, b * t_batch + q0 + 512)
                po = ps_o.tile([65, 512], F32, tag="ps_o")
                nkt = 4 * (qc + 1)
                for kt in range(nkt):
                    k0 = kt * 128
                    # columns below oi*128 of a diagonal tile are fully
                    # causally masked -> skip them in every op of the unit
                    o0 = max(0, k0 - q0)
                    qlive = slice(b * t_batch + q0 + o0,
                                  b * t_batch + q0 + 512)
                    ps = ps_s.tile([128, 512], F32, tag="ps_s")
                    nc.tensor.matmul(
                        ps[:, o0:512],
                        kf[:, b * t_batch + k0: b * t_batch + k0 + 128],
                        qf[:, qlive], start=True, stop=True)
                    if k0 >= q0:      # diagonal region: mask + RPE band
                        oi = (k0 - q0) // 128
                        nc.vector.tensor_add(
                            ps[:, o0:512], ps[:, o0:512],
                            mb[:, oi * 512 + o0:(oi + 1) * 512])
                    elif k0 == q0 - 128:  # band spill from prev chunk
                        nc.vector.tensor_add(ps[:, 0:16], ps[:, 0:16],
                                             spill[:])
                    p_t = ppool.tile([128, 512], BF, tag="p")
                    nc.scalar.activation(p_t[:, o0:512], ps[:, o0:512],
                                         AF.Exp)
                    nc.tensor.matmul(
                        po[:, o0:512],
                        vaug[:, (b * n_kt + kt) * 65:
                             (b * n_kt + kt) * 65 + 65],
                        p_t[:, o0:512], start=(kt == 0),
                        stop=(kt == nkt - 1))
                # normalize: 1/denom row broadcast via K=1 matmul
                den = rows.tile([1, 512], F32, tag="den")
                nc.vector.tensor_copy(den[:], po[64:65, :])
                rden = rows.tile([1, 512], F32, tag="rden")
                nc.vector.reciprocal(rden[:], den[:])
                rb = rows.tile([1, 512], BF, tag="rb")
                nc.vector.tensor_copy(rb[:], rden[:])
                pbc = ps_bc.tile([64, 512], F32, tag="ps_bc")
                nc.tensor.matmul(pbc[:], ones_row[:], rb[:],
                                 start=True, stop=True)
                bc_sb = scratch.tile([64, 512], F32, tag="bc_sb")
                nc.vector.tensor_copy(bc_sb[:], pbc[:])
                nc.vector.tensor_mul(of[:, qcols], po[0:64, :], bc_sb[:])
                # this chunk IS destination core r's payload: stage it now
                nc.sync.dma_start(bounce_o[r], of[:, qcols])

            # ---- AllToAll: deliver all heads' o for own tokens ----
            o_own = dram.tile([n_cores * 64, t_own], BF, tag="a2a_o")
            if not skip_collectives:
                nc.gpsimd.collective_compute(
                    "AllToAll", ALU.bypass, replica_groups=rg,
                    ins=[bounce_o[:].opt()], outs=[o_own[:].opt()])

            # prefetch next layer's weights while the A2A is in flight
            if l + 1 < L:
                weights = load_layer_weights(l + 1)
            else:
                # last layer: prefetch the lm_head weight halves instead,
                # into the idle w1/w2 ring slots
                for h in range(2):
                    wt = wpool.tile([128, CC * (vsh // 2)], BF,
                                    tag=("w1", "w2")[h])
                    nc.gpsimd.dma_start(
                        wt[:].rearrange("p (cc v) -> p cc v", cc=CC),
                        wout_d.ap()[:, :, h * (vsh // 2):
                                    (h + 1) * (vsh // 2)])
                    wout_t[h] = wt

            # o feature-major in SBUF: o_sb[:, cc*t_own + t]
            for tt in range(4):
                nc.sync.dma_start(
                    o_sb[:].rearrange("p (cc t) -> p cc t", cc=CC)
                    [:, :, tt * 128:(tt + 1) * 128],
                    o_own[:].rearrange("(cc p) t -> p cc t", p=128)
                    [:, :, tt * 128:(tt + 1) * 128])

            # ---- Wo projection + residual; transposes deferred so the
            # PE never waits on an LN chain ----
            xlns = []
            for tt in range(4):
                px = ps_proj.tile([128, C], F32, tag="mm")
                for cc in range(CC):
                    nc.tensor.matmul(
                        px[:],
                        o_sb[:, cc * t_own + tt * 128:
                             cc * t_own + (tt + 1) * 128],
                        wo_t[:, cc * C:(cc + 1) * C],
                        start=(cc == 0), stop=(cc == CC - 1))
                xt = x_sb[:, tt * C:(tt + 1) * C]
                nc.vector.scalar_tensor_tensor(xt, px[:], 1.0, xt,
                                               ALU.mult, ALU.add)
                nc.vector.tensor_add(xt, xt, bo_r)
                xln = scratch.tile([128, C], BF, tag=f"ln_out{tt % 2}")
                layernorm(xln[:], xt, g2, be2)
                xlns.append(xln)
                if tt >= 1:   # transpose the PREVIOUS tile (its LN is done)
                    tp = tt - 1
                    for cc in range(CC):
                        transpose_128(
                            xT2[:, cc * t_own + tp * 128:
                                cc * t_own + (tp + 1) * 128],
                            xlns[tp][:, cc * 128:(cc + 1) * 128])
            for cc in range(CC):
                transpose_128(
                    xT2[:, cc * t_own + 3 * 128: cc * t_own + 4 * 128],
                    xlns[3][:, cc * 128:(cc + 1) * 128])

            # ---- FFN + next-layer LN1 + AG, per tile pair ----
            if l + 1 < L:
                g1n, be1n = l1next[:, 0:C], l1next[:, C:2 * C]
            else:
                lnf_t = const.tile([128, 2 * C], BF, tag="lnf")
                nc.sync.dma_start(
                    lnf_t[:].rearrange("p (k c) -> p k c", k=2),
                    ln1_rep.ap()[L])
                g1n, be1n = lnf_t[:, 0:C], lnf_t[:, C:2 * C]

            def ffn1(pair):
                for hs in range(NHS):
                    ph = ps_proj.tile([128, 256], F32, tag="mm")
                    for cc in range(CC):
                        nc.tensor.matmul(
                            ph[:],
                            w1_t[:, cc * HID + hs * 128:
                                 cc * HID + (hs + 1) * 128],
                            xT2[:, cc * t_own + pair * 256:
                                cc * t_own + (pair + 1) * 256],
                            start=(cc == 0), stop=(cc == CC - 1))
                    nc.scalar.activation(
                        actsT[:, hs * t_own + pair * 256:
                              hs * t_own + (pair + 1) * 256],
                        ph[:], AF.Relu, bias=b1_t[:, hs:hs + 1])

            def ffn2(pair):
                for s in range(2):
                    tt = 2 * pair + s
                    pf = ps_proj.tile([128, C], F32, tag="mm")
                    for hs in range(NHS):
                        nc.tensor.matmul(
                            pf[:],
                            actsT[:, hs * t_own + tt * 128:
                                  hs * t_own + (tt + 1) * 128],
                            w2_t[:, hs * C:(hs + 1) * C],
                            start=(hs == 0), stop=(hs == NHS - 1))
                    xt = x_sb[:, tt * C:(tt + 1) * C]
                    nc.vector.scalar_tensor_tensor(xt, pf[:], 1.0, xt,
                                                   ALU.mult, ALU.add)
                    nc.vector.tensor_add(xt, xt, b2_r)

            # emission order keeps the PE busy while LN chains run on
            # ACT/DVE: FFN1(p1) sits between pair-0's LNs and transposes
            ag_nxt = [None, None]
            ffn1(0)
            ffn2(0)
            x0lns = ln1_pair(0, g1n, be1n)
            ffn1(1)
            ag_nxt[0] = transpose_pair_ag(0, x0lns, f"l{l + 1}_")
            ffn2(1)
            ag_nxt[1] = ln1_pair_to_ag(1, g1n, be1n, f"l{l + 1}_")
            ag_cur = ag_nxt

        # ================= lm_head (vocab-sharded) =================
        ps_bc_cm.__exit__(None, None, None)
        ps_o_cm.__exit__(None, None, None)
        ps_s_cm.__exit__(None, None, None)
        ps_y = pool(name="ps_y", bufs=5, space="PSUM")

        for p in range(2):
            for r in range(n_cores):
                xtc = xtc_pool.tile([128, PW], BF, tag="xtc")
                nc.sync.dma_start(xtc[:], ag_cur[p][r])
                for s in range(2):
                    st = 2 * p + s
                    g_t0 = r * t_own + st * 128
                    for vc in range(n_vc):
                        v0 = vc * vc_w
                        h = v0 // (vsh // 2)
                        hv0 = v0 - h * (vsh // 2)
                        hw = vsh // 2
                        py = ps_y.tile([128, vc_w], F32, tag="ps_y")
                        for cc in range(CC):
                            nc.tensor.matmul(
                                py[:],
                                xtc[:, cc * 256 + s * 128:
                                    cc * 256 + (s + 1) * 128],
                                wout_t[h][:, cc * hw + hv0:
                                          cc * hw + hv0 + vc_w],
                                start=(cc == 0), stop=(cc == CC - 1))
                        ysb = ysb_pool.tile([128, vc_w], F16, tag="ysb")
                        if vc % 2 == 0:
                            nc.vector.tensor_copy(ysb[:], py[:])
                        else:
                            nc.scalar.copy(ysb[:], py[:])
                        nc.sync.dma_start(
                            y.ap()[g_t0:g_t0 + 128, v0:v0 + vc_w], ysb[:])

        for cm in reversed(pools):
            cm.__exit__(None, None, None)

    nc.compile()
    return nc


# ======================================================================
# host side
# ======================================================================

def _pack_chunked(w):
    """[C_in, N] -> [128, C_in//128, N]: out[p, cc, n] = w[cc*128 + p, n]."""
    cin, n = w.shape
    return np.ascontiguousarray(w.reshape(cin // 128, 128, n).transpose(1, 0, 2))


def _shard_inputs(inputs, vsh=4000, n_cores=NCORES):
    t_own = T_OWN

    tok = np.asarray(inputs["input_tokens"])
    emb = np.asarray(inputs["tok_emb"], np.float32)
    x0_full = emb[tok]                                   # (B, Tb, C) fp32

    Wq = np.asarray(inputs["Wq"], np.float32)
    Wk = np.asarray(inputs["Wk"], np.float32)
    Wv = np.asarray(inputs["Wv"], np.float32)
    Wo = np.asarray(inputs["Wo"], np.float32)
    W1 = np.asarray(inputs["W1"], np.float32)
    W2 = np.asarray(inputs["W2"], np.float32)
    rpe = np.asarray(inputs["rpe"], np.float32)          # (L, NB, H)
    Wout = np.asarray(inputs["Wout"], np.float32)        # (C, V_tot)
    bo = np.asarray(inputs["bo"], np.float32)
    b1 = np.asarray(inputs["b1"], np.float32)
    b2 = np.asarray(inputs["b2"], np.float32)
    g1 = np.asarray(inputs["ln1_g"], np.float32)
    be1 = np.asarray(inputs["ln1_b"], np.float32)
    g2 = np.asarray(inputs["ln2_g"], np.float32)
    be2 = np.asarray(inputs["ln2_b"], np.float32)
    gf = np.asarray(inputs["lnf_g"], np.float32)
    bef = np.asarray(inputs["lnf_b"], np.float32)

    nL = Wq.shape[0]

    # shared (head/vocab-independent) packs
    wo_p = np.stack([_pack_chunked(Wo[l]) for l in range(nL)]).astype(BF16)
    w1_p = np.stack([_pack_chunked(W1[l]) for l in range(nL)]).astype(BF16)
    w2_p = np.stack([_pack_chunked(W2[l]) for l in range(nL)]).astype(BF16)
    b1t = np.ascontiguousarray(
        b1.reshape(nL, NHS, 128).transpose(0, 2, 1))     # [L,128,NHS]

    rep1 = np.empty((nL + 1, 128, 2, C), np.float32)  # cast to bf16 below
    for l in range(nL):
        rep1[l, :, 0, :] = g1[l][None, :]
        rep1[l, :, 1, :] = be1[l][None, :]
    rep1[nL, :, 0, :] = gf[None, :]
    rep1[nL, :, 1, :] = bef[None, :]
    rep2 = np.empty((nL, 128, 4, C), np.float32)
    for l in range(nL):
        for i, vec in enumerate((g2[l], be2[l], bo[l], b2[l])):
            rep2[l, :, i, :] = vec[None, :]

    # mask+band tiles: mb[l, p, oi, j]; scores sT element (k=q0+oi*128+p,
    # q=q0+j): dqk = j - oi*128 - p
    p_i = np.arange(128)[:, None, None]
    oi_i = np.arange(4)[None, :, None]
    j_i = np.arange(512)[None, None, :]
    dqk = j_i - oi_i * 128 - p_i                         # (128, 4, 512)
    # spill tile: k-tile right before the chunk: k = q0-128+p, q = q0+j2
    p2 = np.arange(128)[:, None]
    j2 = np.arange(16)[None, :]
    dqk2 = j2 + 128 - p2                                 # (128, 16)

    scale = 1.0 / np.sqrt(D)
    in_maps = []
    for r in range(n_cores):
        h = r
        b_idx, blk = divmod(r, 4)
        x0 = np.ascontiguousarray(x0_full[b_idx, blk * t_own:(blk + 1) * t_own])

        # fused q|k stationary [L, 128, CC, 128]; Wq pre-scaled by 1/sqrt(D)
        wqk = np.empty((nL, 128, CC, 128), np.float32)
        wvp = np.empty((nL, 128, CC, D), np.float32)
        for l in range(nL):
            q_sl = Wq[l][:, h * D:(h + 1) * D] * scale   # (C, D)
            k_sl = Wk[l][:, h * D:(h + 1) * D]
            v_sl = Wv[l][:, h * D:(h + 1) * D]
            wqk[l, :, :, 0:64] = q_sl.reshape(CC, 128, D).transpose(1, 0, 2)
            wqk[l, :, :, 64:128] = k_sl.reshape(CC, 128, D).transpose(1, 0, 2)
            wvp[l] = v_sl.reshape(CC, 128, D).transpose(1, 0, 2)

        mb = np.empty((nL, 128, 4, 512), np.float32)
        sp = np.empty((nL, 128, 16), np.float32)
        for l in range(nL):
            delta = rpe[l, :, h] - rpe[l, 0, h]          # (NB,)
            band_val = delta[np.clip(16 - dqk, 0, NB - 1)]
            mb[l] = np.where(dqk < 0, NEG,
                             np.where(dqk <= 16, band_val, 0.0))
            sp[l] = np.where((dqk2 >= 0) & (dqk2 <= 16),
                             delta[np.clip(16 - dqk2, 0, NB - 1)], 0.0)

        wout_sl = Wout[:, r * vsh:(r + 1) * vsh]
        wout_p = _pack_chunked(wout_sl).astype(BF16)

        in_maps.append({
            "x0": x0,
            "wqk": wqk.astype(BF16), "wv": wvp.astype(BF16),
            "wo": wo_p, "w1": w1_p, "w2": w2_p,
            "wout": wout_p,
            "b1t": b1t,
            "ln1_rep": rep1.astype(BF16), "ln2_rep": rep2.astype(BF16),
            "maskband": mb.astype(BF16), "spill": sp.astype(BF16),
        })
    return in_maps


_PROGRAM = None


def _assemble_output(per_core, inputs):
    """per_core: dict name -> [NCORES, ...] stacked per-core outputs."""
    Tb = inputs["input_tokens"].shape[1]
    vsh = V // NCORES
    bout = np.asarray(inputs["bout"], np.float32)
    out = np.empty((B, Tb, V), np.float32)
    for r in range(NCORES):
        yr = np.asarray(per_core["y"][r]).astype(np.float32)  # [B*Tb, vsh]
        yr += bout[r * vsh:(r + 1) * vsh]
        out[:, :, r * vsh:(r + 1) * vsh] = yr.reshape(B, Tb, vsh)
    return out


def kernel(**inputs):
    global _PROGRAM
    if _PROGRAM is None:
        _PROGRAM = build_program()
    in_maps = _shard_inputs(inputs)
    res = run_bass_kernel_spmd(_PROGRAM, in_maps,
                               core_ids=list(range(NCORES)))
    per_core = {"y": [res.results[r]["y"] for r in range(NCORES)]}
    return _assemble_output(per_core, inputs)


# revision 21
# speedup vs baseline: 1.3579x; 1.0889x over previous
"""Trainium2 Bass kernel for a 4-layer GPT language model.

Model: B=2, T=2048, C=512, H=8 heads, L=4 layers, V=32000, relative-position
bias (33 buckets, clip +-16), causal attention, ReLU FFN (4C hidden),
final LN + untied output projection.

Sharding over 8 NeuronCores (one uniform SPMD program; all per-core
differences live in the input data):
 - attention: head-parallel (core r computes head r for all tokens/batches)
 - LN / residual / FFN / Wo projection: token-parallel (core r owns the 512
   contiguous tokens of batch r//4, block r%4)
 - lm_head: vocab-parallel (core r computes Wout columns [4000r, 4000(r+1)))

Pipelining (the point of this version):
 - per layer ONE AllGather of post-LN1 activations, split into two halves
   (token-tile pairs); each half is issued as soon as its pair finishes the
   FFN2+LN1' chain of the previous layer, hiding the ~15us collective
   latency behind the remaining FFN compute.  QKV consumption is split per
   pair so it can start on the first half.
 - attention chunk (b,qc) is emitted immediately after the QKV chunk that
   completes its k/v inputs -> no QKV/attention phase barrier.
 - q|k projections fused into one 128-wide stationary operand (Wq pre-scaled
   by 1/sqrt(D) on host); v stays token-major for the AV matmul.
 - all layer weights are prefetched one layer ahead on the gpsimd DMA path
   (double-buffered rings).
 - lm_head writes y in fp16 (halves the dominant HBM write traffic); bout
   is added on host during unsharding.

All matmul operands bf16 with fp32 PSUM accumulation; LN/softmax/residual in
fp32. Softmax runs without max-subtraction (scores are O(1) at this scale).
Causal mask + RPE bias are applied as one host-precomputed [128,512] add on
the pre-exp scores: -1e30 in the masked triangle, RPE delta vs rpe[l,0,h] in
the 17-wide diagonal band (softmax shift-invariance removes the constant).
The embedding gather, bf16 casts and layout packing happen on host; the host
reassembles the vocab-sharded per-core outputs and adds bout.
"""

import sys

for _p in ("/opt/trn_rl_repo", "/root/.axon_site/_ro/trn_rl_repo"):
    if _p not in sys.path:
        sys.path.append(_p)

import numpy as np
import ml_dtypes

import functools

import concourse.bass as bass
import concourse.bacc as bacc
import concourse.mybir as mybir
import concourse.tile as tile
from concourse import masks
from concourse.bass_utils import run_bass_kernel_spmd



BF16 = ml_dtypes.bfloat16
FP16 = np.float16

# model dims
B, C, H, L, V, MD = 2, 512, 8, 4, 32000, 16
D = C // H          # 64
HID = 4 * C         # 2048
NB = 2 * MD + 1     # 33
NCORES = 8
CC = C // 128       # 4 c-chunks
NHS = HID // 128    # 16 hidden slices
NEG = -1.0e30

F32 = mybir.dt.float32
BF = mybir.dt.bfloat16
F16 = mybir.dt.float16

T_OWN = 512          # tokens owned per core
PW = CC * 256        # AG payload cols per pair (cc-major, 256 tokens)


def build_program(n_cores=NCORES, vsh=4000, skip_collectives=False):
    t_own = T_OWN
    t_batch = 4 * t_own           # 2048
    t_glob = 2 * t_batch          # 4096
    n_qc = 4                      # 512-wide query chunks per batch
    n_kt = 16                     # 128-wide key tiles per batch
    n_vt = 32                     # v tiles (both batches)
    vc_w = 500
    n_vc = vsh // vc_w
    rg = [list(range(n_cores))]
    AF = mybir.ActivationFunctionType
    ALU = mybir.AluOpType

    nc = bacc.Bacc("TRN2", target_bir_lowering=False, debug=False,
                   num_devices=n_cores)

    # ---- per-core external inputs (host pre-packed, see _shard_inputs) ----
    x0 = nc.dram_tensor("x0", [t_own, C], F32, kind="ExternalInput")
    wqk_d = nc.dram_tensor("wqk", [L, 128, CC, 128], BF, kind="ExternalInput")
    wv_d = nc.dram_tensor("wv", [L, 128, CC, D], BF, kind="ExternalInput")
    wo_d = nc.dram_tensor("wo", [L, 128, CC, C], BF, kind="ExternalInput")
    w1_d = nc.dram_tensor("w1", [L, 128, CC, HID], BF, kind="ExternalInput")
    w2_d = nc.dram_tensor("w2", [L, 128, NHS, C], BF, kind="ExternalInput")
    wout_d = nc.dram_tensor("wout", [128, CC, vsh], BF, kind="ExternalInput")
    b1t_d = nc.dram_tensor("b1t", [L, 128, NHS], F32, kind="ExternalInput")
    # replicated per-column vectors: ln1_rep[l] = (g,b) of LN before attn of
    # layer l; slot L holds the final LN.  ln2_rep[l] = (g2, be2, bo, b2).
    ln1_rep = nc.dram_tensor("ln1_rep", [L + 1, 128, 2, C], BF,
                             kind="ExternalInput")
    ln2_rep = nc.dram_tensor("ln2_rep", [L, 128, 4, C], BF,
                             kind="ExternalInput")
    # combined causal-mask + RPE-delta tiles per diagonal offset oi
    mb_d = nc.dram_tensor("maskband", [L, 128, 4, 512], BF,
                          kind="ExternalInput")
    spill_d = nc.dram_tensor("spill", [L, 128, 16], BF, kind="ExternalInput")

    y = nc.dram_tensor("y", [t_glob, vsh], F16, kind="ExternalOutput")

    with tile.TileContext(nc) as tc:
        pools = []

        def pool(**kw):
            cm = tc.tile_pool(**kw)
            p = cm.__enter__()
            pools.append(cm)
            return p

        const = pool(name="const", bufs=1)
        persist = pool(name="persist", bufs=1)
        xtc_pool = pool(name="xtc", bufs=4)   # AG'd xT chunks [128, PW]
        small = pool(name="small", bufs=4)    # [128,1] LN scalars
        rows = pool(name="rows", bufs=2)      # [1,512] denom rows
        scratch = pool(name="scratch", bufs=2)
        ppool = pool(name="ppool", bufs=4)    # P (exp scores) tiles
        ysb_pool = pool(name="ysb", bufs=4)
        lay = pool(name="lay", bufs=1)
        wpool = pool(name="wpool", bufs=2)    # prefetched per-layer weights

        ps_t = pool(name="ps_t", bufs=1, space="PSUM")
        ps_proj = pool(name="ps_proj", bufs=2, space="PSUM")
        ps_s_cm = tc.tile_pool(name="ps_s", bufs=2, space="PSUM")
        ps_s = ps_s_cm.__enter__()
        ps_o_cm = tc.tile_pool(name="ps_o", bufs=2, space="PSUM")
        ps_o = ps_o_cm.__enter__()
        ps_bc_cm = tc.tile_pool(name="ps_bc", bufs=1, space="PSUM")
        ps_bc = ps_bc_cm.__enter__()

        dram = pool(name="dram", bufs=2, space="DRAM")

        # ---------------- constants ----------------
        ident = const.tile([128, 128], BF)
        masks.make_identity(nc, ident[:])
        ones_row = const.tile([1, 64], BF)
        nc.vector.memset(ones_row[:], 1.0)
        eps_t = const.tile([128, 1], F32)
        nc.vector.memset(eps_t[:], 1e-5)

        # residual stream, token-major fp32; tile tt at cols [tt*C,(tt+1)*C)
        x_sb = persist.tile([128, 4 * C], F32)
        for tt in range(4):
            nc.sync.dma_start(
                x_sb[:, tt * C:(tt + 1) * C],
                x0.ap().rearrange("(tt p) c -> p tt c", p=128)[:, tt])

        def layernorm(dst_ap, src_ap, g_ap, b_ap):
            """dst(bf16) = LN(src) with replicated gamma/beta; src [128,C]."""
            ssum = small.tile([128, 1], F32, tag="ln_ssum")
            nc.vector.tensor_reduce(ssum[:], src_ap, mybir.AxisListType.X,
                                    ALU.add)
            nmean = small.tile([128, 1], F32, tag="ln_nmean")
            nc.scalar.mul(nmean[:], ssum[:], -1.0 / C)
            xc = scratch.tile([128, C], F32, tag="ln_xc")
            nc.scalar.activation(xc[:], src_ap, AF.Identity, bias=nmean[:])
            sq = scratch.tile([128, C], F32, tag="ln_sq")
            vsum = small.tile([128, 1], F32, tag="ln_vsum")
            nc.scalar.activation(sq[:], xc[:], AF.Square, accum_out=vsum[:])
            std = small.tile([128, 1], F32, tag="ln_std")
            nc.scalar.activation(std[:], vsum[:], AF.Sqrt, bias=eps_t[:],
                                 scale=1.0 / C)
            rstd = small.tile([128, 1], F32, tag="ln_rstd")
            nc.vector.reciprocal(rstd[:], std[:])
            nc.vector.scalar_tensor_tensor(dst_ap, xc[:], rstd[:], g_ap,
                                           ALU.mult, ALU.mult)
            nc.vector.tensor_add(dst_ap, dst_ap, b_ap)

        def transpose_128(dst_ap, src_ap):
            """dst[128,128] = src[128,128].T via PE; bf16 in/out."""
            pt = ps_t.tile([128, 128], BF, tag="ps_t")
            nc.tensor.transpose(pt[:], src_ap, ident[:])
            nc.vector.tensor_copy(dst_ap, pt[:])

        # xT_own: pair-major [128, 2*PW]; pair p block: cc*256 + (tt%2)*128
        xT_own = lay.tile([128, 2 * PW], BF, tag="xT_own")

        def ln1_pair(pair, g_ap, b_ap):
            """LN the pair's two tiles (ACT/DVE only); returns bf16 tiles."""
            xlns = []
            for s in range(2):
                tt = 2 * pair + s
                xln = scratch.tile([128, C], BF, tag=f"ln_out{s}")
                layernorm(xln[:], x_sb[:, tt * C:(tt + 1) * C], g_ap, b_ap)
                xlns.append(xln)
            return xlns

        def transpose_pair_ag(pair, xlns, tag):
            """Transpose the pair into xT_own, bounce to DRAM, AllGather.

            Returns the shared AG output tile [n_cores, 128, PW]."""
            for s in range(2):
                for cc in range(CC):
                    transpose_128(
                        xT_own[:, pair * PW + cc * 256 + s * 128:
                               pair * PW + cc * 256 + (s + 1) * 128],
                        xlns[s][:, cc * 128:(cc + 1) * 128])
            bounce = dram.tile([128, PW], BF, tag=f"bnc_x{pair}")
            nc.sync.dma_start(bounce[:],
                              xT_own[:, pair * PW:(pair + 1) * PW])
            ag_out = dram.tile([n_cores, 128, PW], BF,
                               addr_space="Shared", tag=f"ag_x{tag}{pair}")
            if not skip_collectives:
                nc.gpsimd.collective_compute(
                    "AllGather", ALU.bypass, replica_groups=rg,
                    ins=[bounce[:].opt()], outs=[ag_out[:].opt()])
            return ag_out

        def ln1_pair_to_ag(pair, g_ap, b_ap, tag):
            return transpose_pair_ag(pair, ln1_pair(pair, g_ap, b_ap), tag)

        def load_layer_weights(l):
            """Prefetch all tensors needed by layer l (gpsimd DMA path).

            Returns dict of SBUF tiles.  With wpool bufs=2 this double-
            buffers against layer l-1's tiles still in use."""
            w = {}
            wqk = wpool.tile([128, CC * 128], BF, tag="wqk")
            nc.gpsimd.dma_start(
                wqk[:].rearrange("p (cc m) -> p cc m", cc=CC), wqk_d.ap()[l])
            wv = wpool.tile([128, CC * D], BF, tag="wv")
            nc.gpsimd.dma_start(
                wv[:].rearrange("p (cc d) -> p cc d", cc=CC), wv_d.ap()[l])
            mb = wpool.tile([128, 4 * 512], BF, tag="mb")
            nc.gpsimd.dma_start(
                mb[:].rearrange("p (oi j) -> p oi j", oi=4), mb_d.ap()[l])
            spill = wpool.tile([128, 16], BF, tag="spill")
            nc.gpsimd.dma_start(spill[:], spill_d.ap()[l])
            wo = wpool.tile([128, CC * C], BF, tag="wo")
            nc.gpsimd.dma_start(
                wo[:].rearrange("p (cc c) -> p cc c", cc=CC), wo_d.ap()[l])
            w1 = wpool.tile([128, CC * HID], BF, tag="w1")
            nc.gpsimd.dma_start(
                w1[:].rearrange("p (cc j) -> p cc j", cc=CC), w1_d.ap()[l])
            w2 = wpool.tile([128, NHS * C], BF, tag="w2")
            nc.gpsimd.dma_start(
                w2[:].rearrange("p (hs c) -> p hs c", hs=NHS), w2_d.ap()[l])
            b1t = wpool.tile([128, NHS], F32, tag="b1t")
            nc.gpsimd.dma_start(b1t[:], b1t_d.ap()[l])
            l2 = wpool.tile([128, 4 * C], BF, tag="l2rep")
            nc.gpsimd.dma_start(
                l2[:].rearrange("p (k c) -> p k c", k=4), ln2_rep.ap()[l])
            l1n = wpool.tile([128, 2 * C], BF, tag="l1rep")
            nc.gpsimd.dma_start(
                l1n[:].rearrange("p (k c) -> p k c", k=2),
                ln1_rep.ap()[l + 1])
            w.update(wqk=wqk, wv=wv, mb=mb, spill=spill, wo=wo, w1=w1,
                     w2=w2, b1t=b1t, l2=l2, l1n=l1n)
            return w

        # ---------------- prologue: LN1 of layer 0 + first AG ----------------
        l1_0 = const.tile([128, 2 * C], BF)
        nc.sync.dma_start(
            l1_0[:].rearrange("p (k c) -> p k c", k=2), ln1_rep.ap()[0])
        weights = load_layer_weights(0)
        ag_cur = [None, None]
        for p in range(2):
            ag_cur[p] = ln1_pair_to_ag(p, l1_0[:, 0:C], l1_0[:, C:2 * C],
                                       "l0_")

        # persistent attention tiles
        qf = lay.tile([64, t_glob], BF, tag="qf")
        kf = lay.tile([64, t_glob], BF, tag="kf")
        vaug = lay.tile([128, n_vt * 65], BF, tag="vaug")
        of = lay.tile([64, t_glob], BF, tag="of")
        o_sb = lay.tile([128, CC * t_own], BF, tag="o_sb")
        xT2 = lay.tile([128, CC * t_own], BF, tag="xT2")
        actsT = lay.tile([128, NHS * t_own], BF, tag="actsT")
        nc.vector.memset(
            vaug[:].rearrange("p (n e) -> p n e", e=65)[:, :, 64:65], 1.0)

        wout_t = [None, None]

        # ================= transformer layers =================
        for l in range(L):
            wqk, wv = weights["wqk"], weights["wv"]
            mb, spill = weights["mb"], weights["spill"]
            wo_t, w1_t, w2_t = weights["wo"], weights["w1"], weights["w2"]
            b1_t, l2rep, l1next = weights["b1t"], weights["l2"], weights["l1n"]
            g2, be2 = l2rep[:, 0:C], l2rep[:, C:2 * C]
            bo_r, b2_r = l2rep[:, 2 * C:3 * C], l2rep[:, 3 * C:4 * C]
            bounce_o = dram.tile([n_cores, 64, t_own], BF, tag="bnc_o")

            # ---- QKV pair-major (pair 0 for all chunks covers AG_1
            # latency), attention chunks interleaved in the pair-1 sweep ----
            def qkv_half(r, p):
                cbase = r * t_own
                xtc = xtc_pool.tile([128, PW], BF, tag="xtc")
                nc.sync.dma_start(xtc[:], ag_cur[p][r])
                pq = ps_proj.tile([128, 256], F32, tag="mm")
                for cc in range(CC):
                    nc.tensor.matmul(
                        pq[:], wqk[:, cc * 128:(cc + 1) * 128],
                        xtc[:, cc * 256:(cc + 1) * 256],
                        start=(cc == 0), stop=(cc == CC - 1))
                cols = slice(cbase + p * 256, cbase + (p + 1) * 256)
                nc.vector.tensor_copy(qf[:, cols], pq[0:64, :])
                nc.vector.tensor_copy(kf[:, cols], pq[64:128, :])
                for s in range(2):
                    pv = ps_proj.tile([128, D], F32, tag="mm")
                    for cc in range(CC):
                        nc.tensor.matmul(
                            pv[:],
                            xtc[:, cc * 256 + s * 128:
                                cc * 256 + (s + 1) * 128],
                            wv[:, cc * D:(cc + 1) * D],
                            start=(cc == 0), stop=(cc == CC - 1))
                    vt = r * 4 + p * 2 + s
                    nc.vector.tensor_copy(
                        vaug[:, vt * 65: vt * 65 + 64], pv[:])

            for r in range(n_cores):
                qkv_half(r, 0)
            for r in range(n_cores):
                qkv_half(r, 1)

                # ---- attention chunk (b, qc) = (r//4, r%4) now ready ----
                b, qc = divmod(r, 4)
                q0 = qc * 512
                qcols = slice(b * t_batch + q0, b * t_batch + q0 + 512)
                po = ps_o.tile([65, 512], F32, tag="ps_o")
                nkt = 4 * (qc + 1)
                for kt in range(nkt):
                    k0 = kt * 128
                    # columns below oi*128 of a diagonal tile are fully
                    # causally masked -> skip them in every op of the unit
                    o0 = max(0, k0 - q0)
                    qlive = slice(b * t_batch + q0 + o0,
                                  b * t_batch + q0 + 512)
                    ps = ps_s.tile([128, 512], F32, tag="ps_s")
                    nc.tensor.matmul(
                        ps[:, o0:512],
                        kf[:, b * t_batch + k0: b * t_batch + k0 + 128],
                        qf[:, qlive], start=True, stop=True)
                    if k0 >= q0:      # diagonal region: mask + RPE band
                        oi = (k0 - q0) // 128
                        nc.vector.tensor_add(
                            ps[:, o0:512], ps[:, o0:512],
                            mb[:, oi * 512 + o0:(oi + 1) * 512])
                    elif k0 == q0 - 128:  # band spill from prev chunk
                        nc.vector.tensor_add(ps[:, 0:16], ps[:, 0:16],
                                             spill[:])
                    p_t = ppool.tile([128, 512], BF, tag="p")
                    nc.scalar.activation(p_t[:, o0:512], ps[:, o0:512],
                                         AF.Exp)
                    nc.tensor.matmul(
                        po[:, o0:512],
                        vaug[:, (b * n_kt + kt) * 65:
                             (b * n_kt + kt) * 65 + 65],
                        p_t[:, o0:512], start=(kt == 0),
                        stop=(kt == nkt - 1))
                # normalize: 1/denom row broadcast via K=1 matmul
                den = rows.tile([1, 512], F32, tag="den")
                nc.vector.tensor_copy(den[:], po[64:65, :])
                rden = rows.tile([1, 512], F32, tag="rden")
                nc.vector.reciprocal(rden[:], den[:])
                rb = rows.tile([1, 512], BF, tag="rb")
                nc.vector.tensor_copy(rb[:], rden[:])
                pbc = ps_bc.tile([64, 512], F32, tag="ps_bc")
                nc.tensor.matmul(pbc[:], ones_row[:], rb[:],
                                 start=True, stop=True)
                bc_sb = scratch.tile([64, 512], F32, tag="bc_sb")
                nc.vector.tensor_copy(bc_sb[:], pbc[:])
                nc.vector.tensor_mul(of[:, qcols], po[0:64, :], bc_sb[:])
                # this chunk IS destination core r's payload: stage it now
                nc.sync.dma_start(bounce_o[r], of[:, qcols])

            # ---- AllToAll: deliver all heads' o for own tokens ----
            o_own = dram.tile([n_cores * 64, t_own], BF, tag="a2a_o")
            if not skip_collectives:
                nc.gpsimd.collective_compute(
                    "AllToAll", ALU.bypass, replica_groups=rg,
                    ins=[bounce_o[:].opt()], outs=[o_own[:].opt()])

            # prefetch next layer's weights while the A2A is in flight
            if l + 1 < L:
                weights = load_layer_weights(l + 1)
            else:
                # last layer: prefetch the lm_head weight halves instead,
                # into the idle w1/w2 ring slots
                for h in range(2):
                    wt = wpool.tile([128, CC * (vsh // 2)], BF,
                                    tag=("w1", "w2")[h])
                    nc.gpsimd.dma_start(
                        wt[:].rearrange("p (cc v) -> p cc v", cc=CC),
                        wout_d.ap()[:, :, h * (vsh // 2):
                                    (h + 1) * (vsh // 2)])
                    wout_t[h] = wt

            # o feature-major in SBUF: o_sb[:, cc*t_own + t]
            for tt in range(4):
                nc.sync.dma_start(
                    o_sb[:].rearrange("p (cc t) -> p cc t", cc=CC)
                    [:, :, tt * 128:(tt + 1) * 128],
                    o_own[:].rearrange("(cc p) t -> p cc t", p=128)
                    [:, :, tt * 128:(tt + 1) * 128])

            # ---- Wo projection + residual; transposes deferred so the
            # PE never waits on an LN chain ----
            xlns = []
            for tt in range(4):
                px = ps_proj.tile([128, C], F32, tag="mm")
                for cc in range(CC):
                    nc.tensor.matmul(
                        px[:],
                        o_sb[:, cc * t_own + tt * 128:
                             cc * t_own + (tt + 1) * 128],
                        wo_t[:, cc * C:(cc + 1) * C],
                        start=(cc == 0), stop=(cc == CC - 1))
                xt = x_sb[:, tt * C:(tt + 1) * C]
                nc.vector.scalar_tensor_tensor(xt, px[:], 1.0, xt,
                                               ALU.mult, ALU.add)
                nc.vector.tensor_add(xt, xt, bo_r)
                xln = scratch.tile([128, C], BF, tag=f"ln_out{tt % 2}")
                layernorm(xln[:], xt, g2, be2)
                xlns.append(xln)
                if tt >= 1:   # transpose the PREVIOUS tile (its LN is done)
                    tp = tt - 1
                    for cc in range(CC):
                        transpose_128(
                            xT2[:, cc * t_own + tp * 128:
                                cc * t_own + (tp + 1) * 128],
                            xlns[tp][:, cc * 128:(cc + 1) * 128])
            for cc in range(CC):
                transpose_128(
                    xT2[:, cc * t_own + 3 * 128: cc * t_own + 4 * 128],
                    xlns[3][:, cc * 128:(cc + 1) * 128])

            # ---- FFN + next-layer LN1 + AG, per tile pair ----
            if l + 1 < L:
                g1n, be1n = l1next[:, 0:C], l1next[:, C:2 * C]
            else:
                lnf_t = const.tile([128, 2 * C], BF, tag="lnf")
                nc.sync.dma_start(
                    lnf_t[:].rearrange("p (k c) -> p k c", k=2),
                    ln1_rep.ap()[L])
                g1n, be1n = lnf_t[:, 0:C], lnf_t[:, C:2 * C]

            def ffn1(pair):
                for hs in range(NHS):
                    ph = ps_proj.tile([128, 256], F32, tag="mm")
                    for cc in range(CC):
                        nc.tensor.matmul(
                            ph[:],
                            w1_t[:, cc * HID + hs * 128:
                                 cc * HID + (hs + 1) * 128],
                            xT2[:, cc * t_own + pair * 256:
                                cc * t_own + (pair + 1) * 256],
                            start=(cc == 0), stop=(cc == CC - 1))
                    nc.scalar.activation(
                        actsT[:, hs * t_own + pair * 256:
                              hs * t_own + (pair + 1) * 256],
                        ph[:], AF.Relu, bias=b1_t[:, hs:hs + 1])

            def ffn2(pair):
                for s in range(2):
                    tt = 2 * pair + s
                    pf = ps_proj.tile([128, C], F32, tag="mm")
                    for hs in range(NHS):
                        nc.tensor.matmul(
                            pf[:],
                            actsT[:, hs * t_own + tt * 128:
                                  hs * t_own + (tt + 1) * 128],
                            w2_t[:, hs * C:(hs + 1) * C],
                            start=(hs == 0), stop=(hs == NHS - 1))
                    xt = x_sb[:, tt * C:(tt + 1) * C]
                    nc.vector.scalar_tensor_tensor(xt, pf[:], 1.0, xt,
                                                   ALU.mult, ALU.add)
                    nc.vector.tensor_add(xt, xt, b2_r)

            # emission order keeps the PE busy while LN chains run on
            # ACT/DVE: FFN1(p1) sits between pair-0's LNs and transposes
            ag_nxt = [None, None]
            ffn1(0)
            ffn2(0)
            x0lns = ln1_pair(0, g1n, be1n)
            ffn1(1)
            ag_nxt[0] = transpose_pair_ag(0, x0lns, f"l{l + 1}_")
            ffn2(1)
            ag_nxt[1] = ln1_pair_to_ag(1, g1n, be1n, f"l{l + 1}_")
            ag_cur = ag_nxt

        # ================= lm_head (vocab-sharded) =================
        ps_bc_cm.__exit__(None, None, None)
        ps_o_cm.__exit__(None, None, None)
        ps_s_cm.__exit__(None, None, None)
        ps_y = pool(name="ps_y", bufs=5, space="PSUM")

        for p in range(2):
            for r in range(n_cores):
                xtc = xtc_pool.tile([128, PW], BF, tag="xtc")
                nc.sync.dma_start(xtc[:], ag_cur[p][r])
                for s in range(2):
                    st = 2 * p + s
                    g_t0 = r * t_own + st * 128
                    for vc in range(n_vc):
                        v0 = vc * vc_w
                        h = v0 // (vsh // 2)
                        hv0 = v0 - h * (vsh // 2)
                        hw = vsh // 2
                        py = ps_y.tile([128, vc_w], F32, tag="ps_y")
                        for cc in range(CC):
                            nc.tensor.matmul(
                                py[:],
                                xtc[:, cc * 256 + s * 128:
                                    cc * 256 + (s + 1) * 128],
                                wout_t[h][:, cc * hw + hv0:
                                          cc * hw + hv0 + vc_w],
                                start=(cc == 0), stop=(cc == CC - 1))
                        ysb = ysb_pool.tile([128, vc_w], F16, tag="ysb")
                        if vc % 2 == 0:
                            nc.vector.tensor_copy(ysb[:], py[:])
                        else:
                            nc.scalar.copy(ysb[:], py[:])
                        nc.sync.dma_start(
                            y.ap()[g_t0:g_t0 + 128, v0:v0 + vc_w], ysb[:])

        for cm in reversed(pools):
            cm.__exit__(None, None, None)

    nc.compile()
    return nc


# ======================================================================
# host side
# ======================================================================

def _pack_chunked(w):
    """[C_in, N] -> [128, C_in//128, N]: out[p, cc, n] = w[cc*128 + p, n]."""
    cin, n = w.shape
    return np.ascontiguousarray(w.reshape(cin // 128, 128, n).transpose(1, 0, 2))


def _shard_inputs(inputs, vsh=4000, n_cores=NCORES):
    t_own = T_OWN

    tok = np.asarray(inputs["input_tokens"])
    emb = np.asarray(inputs["tok_emb"], np.float32)
    x0_full = emb[tok]                                   # (B, Tb, C) fp32

    Wq = np.asarray(inputs["Wq"], np.float32)
    Wk = np.asarray(inputs["Wk"], np.float32)
    Wv = np.asarray(inputs["Wv"], np.float32)
    Wo = np.asarray(inputs["Wo"], np.float32)
    W1 = np.asarray(inputs["W1"], np.float32)
    W2 = np.asarray(inputs["W2"], np.float32)
    rpe = np.asarray(inputs["rpe"], np.float32)          # (L, NB, H)
    Wout = np.asarray(inputs["Wout"], np.float32)        # (C, V_tot)
    bo = np.asarray(inputs["bo"], np.float32)
    b1 = np.asarray(inputs["b1"], np.float32)
    b2 = np.asarray(inputs["b2"], np.float32)
    g1 = np.asarray(inputs["ln1_g"], np.float32)
    be1 = np.asarray(inputs["ln1_b"], np.float32)
    g2 = np.asarray(inputs["ln2_g"], np.float32)
    be2 = np.asarray(inputs["ln2_b"], np.float32)
    gf = np.asarray(inputs["lnf_g"], np.float32)
    bef = np.asarray(inputs["lnf_b"], np.float32)

    nL = Wq.shape[0]

    # shared (head/vocab-independent) packs
    wo_p = np.stack([_pack_chunked(Wo[l]) for l in range(nL)]).astype(BF16)
    w1_p = np.stack([_pack_chunked(W1[l]) for l in range(nL)]).astype(BF16)
    w2_p = np.stack([_pack_chunked(W2[l]) for l in range(nL)]).astype(BF16)
    b1t = np.ascontiguousarray(
        b1.reshape(nL, NHS, 128).transpose(0, 2, 1))     # [L,128,NHS]

    rep1 = np.empty((nL + 1, 128, 2, C), np.float32)  # cast to bf16 below
    for l in range(nL):
        rep1[l, :, 0, :] = g1[l][None, :]
        rep1[l, :, 1, :] = be1[l][None, :]
    rep1[nL, :, 0, :] = gf[None, :]
    rep1[nL, :, 1, :] = bef[None, :]
    rep2 = np.empty((nL, 128, 4, C), np.float32)
    for l in range(nL):
        for i, vec in enumerate((g2[l], be2[l], bo[l], b2[l])):
            rep2[l, :, i, :] = vec[None, :]

    # mask+band tiles: mb[l, p, oi, j]; scores sT element (k=q0+oi*128+p,
    # q=q0+j): dqk = j - oi*128 - p
    p_i = np.arange(128)[:, None, None]
    oi_i = np.arange(4)[None, :, None]
    j_i = np.arange(512)[None, None, :]
    dqk = j_i - oi_i * 128 - p_i                         # (128, 4, 512)
    # spill tile: k-tile right before the chunk: k = q0-128+p, q = q0+j2
    p2 = np.arange(128)[:, None]
    j2 = np.arange(16)[None, :]
    dqk2 = j2 + 128 - p2                                 # (128, 16)

    scale = 1.0 / np.sqrt(D)
    in_maps = []
    for r in range(n_cores):
        h = r
        b_idx, blk = divmod(r, 4)
        x0 = np.ascontiguousarray(x0_full[b_idx, blk * t_own:(blk + 1) * t_own])

        # fused q|k stationary [L, 128, CC, 128]; Wq pre-scaled by 1/sqrt(D)
        wqk = np.empty((nL, 128, CC, 128), np.float32)
        wvp = np.empty((nL, 128, CC, D), np.float32)
        for l in range(nL):
            q_sl = Wq[l][:, h * D:(h + 1) * D] * scale   # (C, D)
            k_sl = Wk[l][:, h * D:(h + 1) * D]
            v_sl = Wv[l][:, h * D:(h + 1) * D]
            wqk[l, :, :, 0:64] = q_sl.reshape(CC, 128, D).transpose(1, 0, 2)
            wqk[l, :, :, 64:128] = k_sl.reshape(CC, 128, D).transpose(1, 0, 2)
            wvp[l] = v_sl.reshape(CC, 128, D).transpose(1, 0, 2)

        mb = np.empty((nL, 128, 4, 512), np.float32)
        sp = np.empty((nL, 128, 16), np.float32)
        for l in range(nL):
            delta = rpe[l, :, h] - rpe[l, 0, h]          # (NB,)
            band_val = delta[np.clip(16 - dqk, 0, NB - 1)]
            mb[l] = np.where(dqk < 0, NEG,
                             np.where(dqk <= 16, band_val, 0.0))
            sp[l] = np.where((dqk2 >= 0) & (dqk2 <= 16),
                             delta[np.clip(16 - dqk2, 0, NB - 1)], 0.0)

        wout_sl = Wout[:, r * vsh:(r + 1) * vsh]
        wout_p = _pack_chunked(wout_sl).astype(BF16)

        in_maps.append({
            "x0": x0,
            "wqk": wqk.astype(BF16), "wv": wvp.astype(BF16),
            "wo": wo_p, "w1": w1_p, "w2": w2_p,
            "wout": wout_p,
            "b1t": b1t,
            "ln1_rep": rep1.astype(BF16), "ln2_rep": rep2.astype(BF16),
            "maskband": mb.astype(BF16), "spill": sp.astype(BF16),
        })
    return in_maps


_PROGRAM = None


def _assemble_output(per_core, inputs):
    """per_core: dict name -> [NCORES, ...] stacked per-core outputs."""
    Tb = inputs["input_tokens"].shape[1]
    vsh = V // NCORES
    bout = np.asarray(inputs["bout"], np.float32)
    out = np.empty((B, Tb, V), np.float32)
    for r in range(NCORES):
        yr = np.asarray(per_core["y"][r]).astype(np.float32)  # [B*Tb, vsh]
        yr += bout[r * vsh:(r + 1) * vsh]
        out[:, :, r * vsh:(r + 1) * vsh] = yr.reshape(B, Tb, vsh)
    return out


def kernel(**inputs):
    global _PROGRAM
    if _PROGRAM is None:
        _PROGRAM = build_program()
    in_maps = _shard_inputs(inputs)
    res = run_bass_kernel_spmd(_PROGRAM, in_maps,
                               core_ids=list(range(NCORES)))
    per_core = {"y": [res.results[r]["y"] for r in range(NCORES)]}
    return _assemble_output(per_core, inputs)
